# revision 29
# baseline (speedup 1.0000x reference)
"""Trainium2 Bass kernel for AttentiveGraphPooling (gnn_message_passing).

Strategy: shard the 4096 graphs across 8 cores (512 graphs each). batch is
sorted, so each core owns a contiguous node range covering whole graphs ->
pooling / gather / GRU are all core-local, no collectives needed.

Per core, graphs go in 4 blocks of 128, each block in 4 windows of 32
graphs. The host pads each window's nodes to a fixed TW node-tiles so the
program is uniform across cores. Pooling uses a [128, 32] gate-weighted
one-hot stationary per node tile (host-prescaled by 1/count so the PSUM
accumulation directly yields the mean); each window accumulates into its
own 32-partition strip of the [128, 256] pooled PSUM tile.

The gate MLP runs feature-major so every matmul has a *block-constant*
stationary operand:
  h1T[f, n] = W1 @ (x + g_b)^T = (fp8 DoubleRow: W1-pairs stationary, x^T
  pair-interleaved moving, N=512 per 4-tile group) + (gather: gw1
  graph-major half-columns stationary, E^T fp8 moving).
relu(h1T) -> SBUF fp8 (Scalar, per PSUM-bank half), then the gate dot
w2 . relu(h1T) runs on the PE (w2 half-columns stationary, N=512) into a
[1, 512] PSUM row; a DVE copy evacuates the *pre-sigmoid* dots to SBUF; a
DRAM bounce (gpsimd-executed DMA) transposes the rows into [128, GB] gate
columns; sigmoid+bias then runs cheaply on the [128, w] columns (Scalar).
Weighted pooling is eg^T @ x on the PE with the 32-wide stationary. The
GRU runs per graph-block with biases folded in via K=1 matmuls.
"""

import os
import sys

import numpy as np

sys.path.insert(0, "/opt/trn_rl_repo")

H = 256
NBLK = 4  # graph blocks per core
GBLK = 128  # graphs per block
NWIN = 4  # windows per block (32 graphs each)
WG = 32  # graphs per window
NUM_TIMESTEPS = 2
GB = 2  # node tiles per gate-pipeline group
PGB = 8  # node tiles per streamed node-major x group


def _build_program(TW, nblk=NBLK):
    """Build the single-core SPMD Bass program. TW = node tiles per window."""
    from contextlib import ExitStack

    import concourse.bass as bass
    import concourse.tile as tile
    from concourse import bacc, mybir

    fp32 = mybir.dt.float32
    bf16 = mybir.dt.bfloat16
    fp8 = mybir.dt.float8e4
    DR = mybir.MatmulPerfMode.DoubleRow

    NT = NWIN * TW  # node tiles per block
    NTP = NT * 128  # padded nodes per block
    GBW = GB * 128

    nc = bacc.Bacc("TRN2", target_bir_lowering=False, debug=False)

    # ---- DRAM parameters (per-core inputs) ----
    x_d = nc.dram_tensor("xk", [nblk * NTP, H], bf16, kind="ExternalInput")
    xt8_d = nc.dram_tensor("xkT8", [nblk, 128, 2, NTP], fp8, kind="ExternalInput")
    e_d = nc.dram_tensor("ech", [nblk, 128, NT, WG], bf16, kind="ExternalInput")
    etj_d = nc.dram_tensor("etoh", [nblk, 128, NT, 128], fp8, kind="ExternalInput")
    w1t_d = nc.dram_tensor("w1t", [2, 128, H], bf16, kind="ExternalInput")
    w1dr_d = nc.dram_tensor("w1dr", [2, 128, 2, 128], fp8, kind="ExternalInput")
    b1r_d = nc.dram_tensor("b1row", [1, H], bf16, kind="ExternalInput")
    w2c_d = nc.dram_tensor("w2dr", [128, 2, 16], fp8, kind="ExternalInput")
    b2c_d = nc.dram_tensor("b2s", [1, 1], fp32, kind="ExternalInput")
    wih_d = nc.dram_tensor("wih_t", [2, 128, 3 * H], bf16, kind="ExternalInput")
    whh_d = nc.dram_tensor("whh_t", [2, 128, 3 * H], bf16, kind="ExternalInput")
    brz_d = nc.dram_tensor("bsum_rz", [1, 2 * H], bf16, kind="ExternalInput")
    bin_d = nc.dram_tensor("bihn", [1, H], bf16, kind="ExternalInput")
    bhn_d = nc.dram_tensor("bhhn", [1, H], bf16, kind="ExternalInput")
    eye_d = nc.dram_tensor("eye128", [128, 128], fp32, kind="ExternalInput")
    out_d = nc.dram_tensor("out", [nblk * GBLK, H], fp32, kind="ExternalOutput")

    with tile.TileContext(nc) as tc, ExitStack() as ctx:
        ep = ctx.enter_context  # shorthand

        const = ep(tc.tile_pool(name="const", bufs=1))
        eres = ep(tc.tile_pool(name="eres", bufs=2))
        etres = ep(tc.tile_pool(name="etres", bufs=2))
        xtstr = ep(tc.tile_pool(name="xtstr", bufs=4))
        xnstr = ep(tc.tile_pool(name="xnstr", bufs=6))
        xastr = ep(tc.tile_pool(name="xastr", bufs=4))
        epool = ep(tc.tile_pool(name="egpool", bufs=8))
        trsh = ep(tc.tile_pool(name="trsh", bufs=3))
        growp = ep(tc.tile_pool(name="grow", bufs=3))
        gtp = ep(tc.tile_pool(name="gtp", bufs=2))
        gsb = ep(tc.tile_pool(name="gsb", bufs=3))
        smallsb = ep(tc.tile_pool(name="smallsb", bufs=2))

        gscrp = ep(tc.tile_pool(name="gscr", bufs=4, space="DRAM"))

        psp = ep(tc.tile_pool(name="psp", bufs=2, space="PSUM"))

        # ---- load constants ----
        def cload(shape, src, tag, dt=fp32):
            t = const.tile(shape, dt, tag=tag)
            nc.sync.dma_start(t[:], src)
            return t

        eye = cload([128, 128], eye_d[:], "c_eye")
        w1t = [cload([128, H], w1t_d[k], f"c_w1t{k}", bf16) for k in range(2)]
        w1dr = [cload([128, 2, 128], w1dr_d[h], f"c_w1dr{h}", fp8)
                for h in range(2)]
        b1row = cload([1, H], b1r_d[:], "c_b1r", bf16)
        w2dr = cload([128, 2, 16], w2c_d[:], "c_w2dr", fp8)
        b2s = cload([1, 1], b2c_d[:], "c_b2s")
        wih = [cload([128, 3 * H], wih_d[k], f"c_wih{k}", bf16) for k in range(2)]
        whh = [cload([128, 3 * H], whh_d[k], f"c_whh{k}", bf16) for k in range(2)]
        brz = cload([1, 2 * H], brz_d[:], "c_brz", bf16)
        bin_ = cload([1, H], bin_d[:], "c_bin", bf16)
        bhn = cload([1, H], bhn_d[:], "c_bhn", bf16)
        ones_row = const.tile([1, 128], bf16)
        nc.vector.memset(ones_row[:], 1.0)

        def fm_copy(g_ap, pool, tag, dt):
            """(128,256) graph-major -> feature-major (128,2,128) via PE."""
            gf = pool.tile([128, 2, GBLK], dt, tag=tag)
            for ki in range(2):
                tp = psp.tile([128, 128], fp32, tag="psrow", name="tp")
                nc.tensor.matmul(tp[:], g_ap[:, ki * 128 : (ki + 1) * 128], eye[:],
                                 is_transpose=True, start=True, stop=True)
                nc.scalar.copy(gf[:, ki, :], tp[:])
            return gf

        blocks = []

        def load_block(j):
            """DMA one-hot chunks for block j."""
            ej = eres.tile([128, NT, WG], bf16, tag="eres", name="ej")
            nc.sync.dma_start(ej[:], e_d[j])
            etj = etres.tile([128, NT, 128], fp8, tag="etres", name="etj")
            nc.sync.dma_start(etj[:], etj_d[j])
            return ej, etj

        def xn_stream(j, t0, n, pool, tag):
            """Stream node-major x tiles [t0, t0+n) of block j."""
            xn = pool.tile([128, PGB, H], bf16, tag=tag, name="xn")
            base = j * NTP + t0 * 128
            nc.sync.dma_start(
                xn[:, 0:n, :], x_d[base : base + n * 128, :].rearrange(
                    "(c p) h -> p c h", p=128
                )
            )
            return xn

        blocks.append(load_block(0))
        for j in range(nblk):
            ej, etj = blocks[j]

            def phase_a(j, ej):
                """Initial mean pool for block j -> (g_gm, g_fm)."""
                pooled = psp.tile([GBLK, H], fp32, tag="pspool")
                for t0 in range(0, NT, PGB):
                    na = min(PGB, NT - t0)
                    xn = xn_stream(j, t0, na, xastr, "xastr")
                    for c in range(na):
                        t = t0 + c
                        w = t // TW
                        nc.tensor.matmul(
                            pooled[w * WG : (w + 1) * WG, :],
                            ej[:, t, :], xn[:, c, :],
                            start=(t % TW == 0), stop=(t % TW == TW - 1),
                            skip_group_check=True, tile_position=(0, w * WG),
                        )
                g_gm = gsb.tile([GBLK, H], fp32, tag="gsb")
                nc.scalar.copy(g_gm[:], pooled[:])
                return g_gm, fm_copy(g_gm[:], gsb, "gfm", bf16)

            if j == 0:
                g_gm, g_fm = phase_a(0, ej)
            else:
                g_gm, g_fm = ga_next

            # ---- timesteps ----
            for ts in range(NUM_TIMESTEPS):
                if ts == 0 and j + 1 < nblk:
                    # prefetch next block's one-hots and issue its phase A
                    # up front: its pool matmuls are DMA-paced, so they
                    # trickle into this block's PE drain gaps, and the
                    # pspool slot rotation leaves hnp an early slot
                    blocks.append(load_block(j + 1))
                    ga_next = phase_a(j + 1, blocks[j + 1][0])
                # GW1 = G @ W1^T + b1  (graph-major, bf16) — also the
                # stationary operand of the gather matmuls
                gw1p = psp.tile([GBLK, H], fp32, tag="psrow", name="gw1p")
                for ki in range(2):
                    nc.tensor.matmul(gw1p[:], g_fm[:, ki, :], w1t[ki][:],
                                     start=(ki == 0), stop=False,
                                     skip_group_check=True)
                nc.tensor.matmul(gw1p[:], ones_row[:], b1row[:],
                                 start=False, stop=True, skip_group_check=True)
                gw1 = gsb.tile([GBLK, H], bf16, tag="gw1")
                nc.scalar.copy(gw1[:], gw1p[:])

                pooled = psp.tile([GBLK, H], fp32, tag="pspool")
                gtall = gtp.tile([128, NT], fp32, tag="gtall")
                ng = NT // GB

                def pool_pair(pp):
                    """eg = ech * gate ; pooled += eg^T @ x for pair pp."""
                    t0 = pp * PGB
                    n = min(PGB, NT - t0)
                    xn = xn_stream(j, t0, n, xnstr, "xnstr")
                    # one batched DVE multiply for the whole pair: the gate
                    # column broadcasts over the 32 window columns
                    eg = epool.tile([128, PGB, WG], bf16, tag="eg")
                    nc.vector.tensor_tensor(
                        eg[:, 0:n, :], ej[:, t0 : t0 + n, :],
                        gtall[:, t0 : t0 + n].unsqueeze(-1).broadcast_to(
                            [128, n, WG]),
                        op=mybir.AluOpType.mult,
                    )
                    for c in range(n):
                        t = t0 + c
                        w = t // TW
                        nc.tensor.matmul(
                            pooled[w * WG : (w + 1) * WG, :], eg[:, c, :],
                            xn[:, c, :],
                            start=(t % TW == 0), stop=(t % TW == TW - 1),
                            skip_group_check=True, tile_position=(0, w * WG),
                        )

                # gate phase: group pairs with stationary-operand reuse;
                # pooling MMs interleave LAG pairs behind so the gate tail
                # (dot -> bounce -> sigmoid) never stalls PE
                LAG = 3
                for p0 in range(0, ng, 2):
                    gis = [gi for gi in (p0, p0 + 1) if gi < ng]
                    xts_l, h1t_l = [], []
                    for gi in gis:
                        xts = xtstr.tile([128, 2, GBW], fp8, tag="xtstr")
                        nc.sync.dma_start(
                            xts[:],
                            xt8_d[j, :, :, gi * GBW : (gi + 1) * GBW],
                        )
                        xts_l.append(xts)
                        h1t_l.append(
                            psp.tile([128, 2, GBW], fp32, tag="h1t",
                                     name="h1t", bufs=4)
                        )
                    for h in range(2):
                        for k in range(len(gis)):
                            nc.tensor.matmul(h1t_l[k][:, h, :], w1dr[h][:],
                                             xts_l[k][:], start=True,
                                             stop=False, perf_mode=DR,
                                             skip_group_check=True)
                        for k, gi in enumerate(gis):
                            nc.tensor.matmul(
                                h1t_l[k][:, h, :],
                                gw1[:, h * 128 : (h + 1) * 128],
                                etj[:, gi * GB : (gi + 1) * GB, :],
                                start=False, stop=True,
                                skip_group_check=True,
                            )
                    for k, gi in enumerate(gis):
                        rsb = trsh.tile([128, 2, GBW], fp8, tag="trsh")
                        if k == 0:
                            # relu on Scalar for group 0 of the pair
                            nc.scalar.activation(
                                rsb[:], h1t_l[k][:],
                                mybir.ActivationFunctionType.Relu)
                        else:
                            # ... and on Vector for group 1
                            nc.vector.tensor_scalar(
                                rsb[:], h1t_l[k][:], 0.0, None,
                                op0=mybir.AluOpType.max)
                        s_ps = psp.tile([16, GB, 128], fp32, tag="psrow")
                        nc.tensor.matmul(s_ps[:], w2dr[:], rsb[:],
                                         start=True, stop=True, perf_mode=DR)
                        if k == 0:
                            g_row = growp.tile([1, 2, GB, 128], fp32,
                                               tag="grow")
                        # fused sigmoid+bias evacuation of the dots (Scalar)
                        nc.scalar.activation(
                            g_row[:, k, :, :], s_ps[0:1, :, :],
                            mybir.ActivationFunctionType.Sigmoid, bias=b2s[:])
                    # one DRAM bounce per pair (gpsimd DMA) transposes
                    # both gate rows into per-tile columns
                    npair = len(gis)
                    gscr = gscrp.tile([npair, GB * 128], fp32, tag="gscr",
                                      name="gscr")
                    nc.gpsimd.dma_start(gscr[:], g_row[:, :npair, :, :])
                    nc.gpsimd.dma_start(
                        gtall[:, p0 * GB : (p0 + npair) * GB],
                        gscr[:].rearrange("g (c n) -> n (g c)", n=128),
                    )
                    # two gate pairs make one 8-tile pool chunk
                    if p0 % 4 == 2:
                        pp = (p0 - 2) // 4
                        if pp >= LAG:
                            pool_pair(pp - LAG)
                npc = (NT + PGB - 1) // PGB
                for pp in range(max(0, npc - LAG), npc):
                    pool_pair(pp)
                ps_sb = gsb.tile([GBLK, H], fp32, tag="poolsb")
                nc.scalar.copy(ps_sb[:], pooled[:])
                pf = fm_copy(ps_sb[:], gsb, "poolfm", bf16)

                # ---- GRU cell (graph-major) ----
                gf, h_old = g_fm, g_gm

                def gru_mm(psum, wi, wh, bias_row, bcol0, bn):
                    mms = []
                    if wi is not None:
                        mms += [(pf[:, ki, :], wi[ki][:, bcol0 : bcol0 + bn])
                                for ki in range(2)]
                    if wh is not None:
                        mms += [(gf[:, ki, :], wh[ki][:, bcol0 : bcol0 + bn])
                                for ki in range(2)]
                    for i, (lhsT, rhs) in enumerate(mms):
                        nc.tensor.matmul(
                            psum[:], lhsT, rhs, start=(i == 0), stop=False,
                            skip_group_check=True,
                        )
                    nc.tensor.matmul(
                        psum[:], ones_row[:], bias_row, start=False, stop=True,
                        skip_group_check=True,
                    )

                # hn first: it only needs gf (ready at ts start), so its MMs
                # can overlap the pool drain
                hnp = psp.tile([GBLK, H], fp32, tag="pspool", name="hnp")
                gru_mm(hnp, None, whh, bhn[:], 2 * H, H)
                rp = psp.tile([GBLK, H], fp32, tag="pspool", name="rp")
                gru_mm(rp, wih, whh, brz[:, 0:H], 0, H)
                r = smallsb.tile([GBLK, H], fp32, tag="gru_r")
                nc.scalar.activation(r[:], rp[:], mybir.ActivationFunctionType.Sigmoid)
                t1 = smallsb.tile([GBLK, H], fp32, tag="gru_s1")
                nc.vector.tensor_mul(t1[:], r[:], hnp[:])
                zp = psp.tile([GBLK, H], fp32, tag="pspool", name="zp")
                gru_mm(zp, wih, whh, brz[:, H : 2 * H], H, H)
                z = smallsb.tile([GBLK, H], fp32, tag="gru_z")
                nc.scalar.activation(z[:], zp[:], mybir.ActivationFunctionType.Sigmoid)
                inp_ = psp.tile([GBLK, H], fp32, tag="pspool", name="inp_")
                gru_mm(inp_, wih, None, bin_[:], 2 * H, H)
                t2 = smallsb.tile([GBLK, H], fp32, tag="gru_s2")
                nc.vector.tensor_add(t2[:], t1[:], inp_[:])
                n = smallsb.tile([GBLK, H], fp32, tag="gru_n")
                nc.scalar.activation(n[:], t2[:], mybir.ActivationFunctionType.Tanh)
                t3 = smallsb.tile([GBLK, H], fp32, tag="gru_s1")
                nc.vector.tensor_sub(t3[:], h_old[:], n[:])
                t4 = smallsb.tile([GBLK, H], fp32, tag="gru_s2")
                nc.vector.tensor_mul(t4[:], z[:], t3[:])
                t5 = smallsb.tile([GBLK, H], fp32, tag="gru_s3")
                nc.vector.tensor_add(t5[:], n[:], t4[:])
                g_gm = gsb.tile([GBLK, H], fp32, tag="gsb")
                nc.scalar.activation(g_gm[:], t5[:],
                                     mybir.ActivationFunctionType.Relu)
                if ts < NUM_TIMESTEPS - 1:
                    g_fm = fm_copy(g_gm[:], gsb, "gfm", bf16)

            nc.sync.dma_start(out_d[j * GBLK : (j + 1) * GBLK, :], g_gm[:])

    nc.compile()
    return nc


def _prep_inputs(x, batch, counts, n_cores, nblk, TW=None):
    """Host-side shard + window-pad + layout. Returns (per_core, TW)."""
    import ml_dtypes

    G = n_cores * nblk * GBLK
    NWTOT = G // WG
    batch = np.asarray(batch).astype(np.int64)
    x = np.asarray(x, dtype=np.float32)

    win_edges = np.searchsorted(batch, np.arange(0, G + 1, WG))
    win_cnt = np.diff(win_edges)
    if TW is None:
        TW = int(np.ceil(win_cnt.max() / 128))
    NT = NWIN * TW
    NTP = NT * 128
    TWP = TW * 128  # padded nodes per window

    invc_all = (1.0 / np.maximum(counts, 1.0)).astype(np.float32)

    xb = x.astype(ml_dtypes.bfloat16)
    gar = np.arange(WG, dtype=np.int64)
    per_core = []
    for k in range(n_cores):
        xk = np.zeros((nblk * NTP, H), dtype=ml_dtypes.bfloat16)
        ech = np.zeros((nblk, 128, NT, WG), dtype=ml_dtypes.bfloat16)
        etoh = np.zeros((nblk, 128, NT, 128), dtype=ml_dtypes.float8_e4m3)
        for j in range(nblk):
            bi = k * nblk + j
            lb = np.full(NTP, -1, dtype=np.int64)
            for w in range(NWIN):
                W = bi * NWIN + w
                lo, hi = win_edges[W], win_edges[W + 1]
                cnt = hi - lo
                base = j * NTP + w * TWP
                xk[base : base + cnt] = xb[lo:hi]
                lb[w * TWP : w * TWP + cnt] = batch[lo:hi] - (bi * GBLK)
            lt = lb.reshape(NT, 128)
            # window-local one-hot scaled by 1/count: ech[p, t, c]
            wof = (np.arange(NT) // TW) * WG  # window col offset per tile
            m = lt[:, :, None] == (wof[:, None, None] + gar[None, None, :])
            vals = invc_all[bi * GBLK + np.clip(lt, 0, GBLK - 1)]
            ech[j] = (m * vals[:, :, None]).transpose(1, 0, 2).astype(
                ml_dtypes.bfloat16)
            # full-block one-hot transpose for the gather matmuls
            e = (lt[:, :, None] == np.arange(GBLK)[None, None, :])
            etoh[j] = e.transpose(2, 0, 1).astype(ml_dtypes.float8_e4m3)
        # pair-interleaved feature-major fp8: [p, ki, node]
        xkT8 = np.ascontiguousarray(
            xk.reshape(nblk, NTP, 2, 128).transpose(0, 3, 2, 1)
        ).astype(ml_dtypes.float8_e4m3)
        per_core.append({"xk": xk, "xkT8": xkT8, "ech": ech, "etoh": etoh})
    return per_core, TW


def _const_inputs(gate_w1, gate_b1, gate_w2, gate_b2, gru_w_ih, gru_w_hh,
                  gru_b_ih, gru_b_hh):
    import ml_dtypes

    f = np.float32
    bf = ml_dtypes.bfloat16
    f8 = ml_dtypes.float8_e4m3
    c = {}
    w1 = np.asarray(gate_w1, f)
    c["w1t"] = np.ascontiguousarray(w1.T.reshape(2, 128, H)).astype(bf)
    # DoubleRow stationary: w1dr[h, p, i, m] = W1[h*128+m, i*128+p]
    c["w1dr"] = np.ascontiguousarray(
        w1.T.reshape(2, 128, 2, 128).transpose(2, 1, 0, 3)).astype(f8)
    c["b1row"] = np.asarray(gate_b1, f).reshape(1, H).astype(bf)
    w2p = np.zeros((128, 2, 16), f)
    w2p[:, :, 0] = np.asarray(gate_w2, f).reshape(2, 128).T
    c["w2dr"] = w2p.astype(f8)
    c["b2s"] = np.asarray(gate_b2, f).reshape(1, 1)
    c["wih_t"] = np.ascontiguousarray(
        np.asarray(gru_w_ih, f).T).reshape(2, 128, 3 * H).astype(bf)
    c["whh_t"] = np.ascontiguousarray(
        np.asarray(gru_w_hh, f).T).reshape(2, 128, 3 * H).astype(bf)
    bih = np.asarray(gru_b_ih, f)
    bhh = np.asarray(gru_b_hh, f)
    c["bsum_rz"] = (bih[: 2 * H] + bhh[: 2 * H]).reshape(1, 2 * H).astype(bf)
    c["bihn"] = bih[2 * H :].reshape(1, H).astype(bf)
    c["bhhn"] = bhh[2 * H :].reshape(1, H).astype(bf)
    c["eye128"] = np.eye(128, dtype=f)
    return c


_CACHE = {}


def run(x, gate_w1, gate_b1, gate_w2, gate_b2, gru_w_ih, gru_w_hh, gru_b_ih,
        gru_b_hh, batch, num_graphs, n_cores=8, nblk=NBLK, trace=False,
        use_sim=False):
    from concourse.bass_utils import run_bass_kernel_spmd

    batch = np.asarray(batch).astype(np.int64)
    G = n_cores * nblk * GBLK
    counts = np.bincount(batch, minlength=G).astype(np.float32)
    per_core, TW = _prep_inputs(x, batch, counts, n_cores, nblk)
    consts = _const_inputs(gate_w1, gate_b1, gate_w2, gate_b2, gru_w_ih,
                           gru_w_hh, gru_b_ih, gru_b_hh)
    in_maps = [{**consts, **pc} for pc in per_core]

    key = (TW, nblk, n_cores)
    if key not in _CACHE:
        _CACHE[key] = _build_program(TW, nblk=nblk)
    nc = _CACHE[key]

    if use_sim:
        from concourse.bass_interp import CoreSim

        outs = []
        for k in range(n_cores):
            sim = CoreSim(nc)
            for name, arr in in_maps[k].items():
                sim.tensor(name)[:] = arr
            sim.simulate()
            outs.append(np.array(sim.tensor("out")))
        return np.concatenate(outs, axis=0), None

    res = run_bass_kernel_spmd(nc, in_maps, core_ids=list(range(n_cores)),
                               trace=trace)
    out = np.concatenate([res.results[k]["out"] for k in range(n_cores)], axis=0)
    return out, res


def kernel(**inputs):
    out, _ = run(**inputs)
    return out


# revision 30
# speedup vs baseline: 1.1519x; 1.1519x over previous
"""Trainium2 Bass kernel for AttentiveGraphPooling (gnn_message_passing).

Strategy: shard the 4096 graphs across 8 cores (512 graphs each). batch is
sorted, so each core owns a contiguous node range covering whole graphs ->
pooling / gather / GRU are all core-local, no collectives needed.

Per core, graphs go in 4 blocks of 128, each block in 4 windows of 32
graphs. The host pads each window's nodes to a fixed TW node-tiles so the
program is uniform across cores. Pooling uses a [128, 32] gate-weighted
one-hot stationary per node tile (host-prescaled by 1/count so the PSUM
accumulation directly yields the mean); each window accumulates into its
own 32-partition strip of the [128, 256] pooled PSUM tile.

The gate MLP runs feature-major so every matmul has a *block-constant*
stationary operand:
  h1T[f, n] = W1 @ (x + g_b)^T = (fp8 DoubleRow: W1-pairs stationary, x^T
  pair-interleaved moving, N=512 per 4-tile group) + (gather: gw1
  graph-major half-columns stationary, E^T fp8 moving).
relu(h1T) -> SBUF fp8 (Scalar, per PSUM-bank half), then the gate dot
w2 . relu(h1T) runs on the PE (w2 half-columns stationary, N=512) into a
[1, 512] PSUM row; a DVE copy evacuates the *pre-sigmoid* dots to SBUF; a
DRAM bounce (gpsimd-executed DMA) transposes the rows into [128, GB] gate
columns; sigmoid+bias then runs cheaply on the [128, w] columns (Scalar).
Weighted pooling is eg^T @ x on the PE with the 32-wide stationary. The
GRU runs per graph-block with biases folded in via K=1 matmuls.
"""

import os
import sys

import numpy as np

sys.path.insert(0, "/opt/trn_rl_repo")

H = 256
NBLK = 4  # graph blocks per core
GBLK = 128  # graphs per block
NWIN = 4  # windows per block (32 graphs each)
WG = 32  # graphs per window
NUM_TIMESTEPS = 2
GB = 4  # node tiles per gate-pipeline group
PGB = 8  # node tiles per streamed node-major x group


def _build_program(TW, nblk=NBLK):
    """Build the single-core SPMD Bass program. TW = node tiles per window."""
    from contextlib import ExitStack

    import concourse.bass as bass
    import concourse.tile as tile
    from concourse import bacc, mybir

    fp32 = mybir.dt.float32
    bf16 = mybir.dt.bfloat16
    fp8 = mybir.dt.float8e4
    DR = mybir.MatmulPerfMode.DoubleRow

    NT = NWIN * TW  # node tiles per block
    NTP = NT * 128  # padded nodes per block
    GBW = GB * 128

    nc = bacc.Bacc("TRN2", target_bir_lowering=False, debug=False)

    # ---- DRAM parameters (per-core inputs) ----
    x_d = nc.dram_tensor("xk", [nblk * NTP, H], bf16, kind="ExternalInput")
    xt8_d = nc.dram_tensor("xkT8", [nblk, 128, 2, NTP], fp8, kind="ExternalInput")
    e_d = nc.dram_tensor("ech", [nblk, 128, NT, WG], bf16, kind="ExternalInput")
    etj_d = nc.dram_tensor("etoh", [nblk, 128, NT, 128], fp8, kind="ExternalInput")
    w1t_d = nc.dram_tensor("w1t", [2, 128, H], bf16, kind="ExternalInput")
    w1dr_d = nc.dram_tensor("w1dr", [2, 128, 2, 128], fp8, kind="ExternalInput")
    b1r_d = nc.dram_tensor("b1row", [1, H], bf16, kind="ExternalInput")
    w2c_d = nc.dram_tensor("w2dr", [128, 2, 16], fp8, kind="ExternalInput")
    b2c_d = nc.dram_tensor("b2s", [1, 1], fp32, kind="ExternalInput")
    wih_d = nc.dram_tensor("wih_t", [2, 128, 3 * H], bf16, kind="ExternalInput")
    whh_d = nc.dram_tensor("whh_t", [2, 128, 3 * H], bf16, kind="ExternalInput")
    brz_d = nc.dram_tensor("bsum_rz", [1, 2 * H], bf16, kind="ExternalInput")
    bin_d = nc.dram_tensor("bihn", [1, H], bf16, kind="ExternalInput")
    bhn_d = nc.dram_tensor("bhhn", [1, H], bf16, kind="ExternalInput")
    eye_d = nc.dram_tensor("eye128", [128, 128], fp32, kind="ExternalInput")
    out_d = nc.dram_tensor("out", [nblk * GBLK, H], fp32, kind="ExternalOutput")

    with tile.TileContext(nc) as tc, ExitStack() as ctx:
        ep = ctx.enter_context  # shorthand

        const = ep(tc.tile_pool(name="const", bufs=1))
        eres = ep(tc.tile_pool(name="eres", bufs=2))
        etres = ep(tc.tile_pool(name="etres", bufs=2))
        xtstr = ep(tc.tile_pool(name="xtstr", bufs=4))
        xnstr = ep(tc.tile_pool(name="xnstr", bufs=6))
        xastr = ep(tc.tile_pool(name="xastr", bufs=4))
        epool = ep(tc.tile_pool(name="egpool", bufs=8))
        trsh = ep(tc.tile_pool(name="trsh", bufs=3))
        growp = ep(tc.tile_pool(name="grow", bufs=3))
        gtp = ep(tc.tile_pool(name="gtp", bufs=2))
        gsb = ep(tc.tile_pool(name="gsb", bufs=3))
        smallsb = ep(tc.tile_pool(name="smallsb", bufs=2))

        gscrp = ep(tc.tile_pool(name="gscr", bufs=4, space="DRAM"))

        psp = ep(tc.tile_pool(name="psp", bufs=2, space="PSUM"))

        # ---- load constants ----
        def cload(shape, src, tag, dt=fp32):
            t = const.tile(shape, dt, tag=tag)
            nc.sync.dma_start(t[:], src)
            return t

        eye = cload([128, 128], eye_d[:], "c_eye")
        w1t = [cload([128, H], w1t_d[k], f"c_w1t{k}", bf16) for k in range(2)]
        w1dr = [cload([128, 2, 128], w1dr_d[h], f"c_w1dr{h}", fp8)
                for h in range(2)]
        b1row = cload([1, H], b1r_d[:], "c_b1r", bf16)
        w2dr = cload([128, 2, 16], w2c_d[:], "c_w2dr", fp8)
        b2s = cload([1, 1], b2c_d[:], "c_b2s")
        wih = [cload([128, 3 * H], wih_d[k], f"c_wih{k}", bf16) for k in range(2)]
        whh = [cload([128, 3 * H], whh_d[k], f"c_whh{k}", bf16) for k in range(2)]
        brz = cload([1, 2 * H], brz_d[:], "c_brz", bf16)
        bin_ = cload([1, H], bin_d[:], "c_bin", bf16)
        bhn = cload([1, H], bhn_d[:], "c_bhn", bf16)
        ones_row = const.tile([1, 128], bf16)
        nc.vector.memset(ones_row[:], 1.0)

        def fm_copy(g_ap, pool, tag, dt):
            """(128,256) graph-major -> feature-major (128,2,128) via PE."""
            gf = pool.tile([128, 2, GBLK], dt, tag=tag)
            for ki in range(2):
                tp = psp.tile([128, 128], fp32, tag="psrow", name="tp")
                nc.tensor.matmul(tp[:], g_ap[:, ki * 128 : (ki + 1) * 128], eye[:],
                                 is_transpose=True, start=True, stop=True)
                nc.scalar.copy(gf[:, ki, :], tp[:])
            return gf

        blocks = []

        def load_block(j):
            """DMA one-hot chunks for block j."""
            ej = eres.tile([128, NT, WG], bf16, tag="eres", name="ej")
            nc.sync.dma_start(ej[:], e_d[j])
            etj = etres.tile([128, NT, 128], fp8, tag="etres", name="etj")
            nc.sync.dma_start(etj[:], etj_d[j])
            return ej, etj

        def xn_stream(j, t0, n, pool, tag):
            """Stream node-major x tiles [t0, t0+n) of block j."""
            xn = pool.tile([128, PGB, H], bf16, tag=tag, name="xn")
            base = j * NTP + t0 * 128
            nc.sync.dma_start(
                xn[:, 0:n, :], x_d[base : base + n * 128, :].rearrange(
                    "(c p) h -> p c h", p=128
                )
            )
            return xn

        blocks.append(load_block(0))
        for j in range(nblk):
            ej, etj = blocks[j]

            def phase_a(j, ej):
                """Initial mean pool for block j -> (g_gm, g_fm)."""
                pooled = psp.tile([GBLK, H], fp32, tag="pspool")
                for t0 in range(0, NT, PGB):
                    na = min(PGB, NT - t0)
                    xn = xn_stream(j, t0, na, xastr, "xastr")
                    for c in range(na):
                        t = t0 + c
                        w = t // TW
                        nc.tensor.matmul(
                            pooled[w * WG : (w + 1) * WG, :],
                            ej[:, t, :], xn[:, c, :],
                            start=(t % TW == 0), stop=(t % TW == TW - 1),
                            skip_group_check=True, tile_position=(0, w * WG),
                        )
                g_gm = gsb.tile([GBLK, H], fp32, tag="gsb")
                nc.scalar.copy(g_gm[:], pooled[:])
                return g_gm, fm_copy(g_gm[:], gsb, "gfm", bf16)

            if j == 0:
                g_gm, g_fm = phase_a(0, ej)
            else:
                g_gm, g_fm = ga_next

            # ---- timesteps ----
            for ts in range(NUM_TIMESTEPS):
                if ts == 0 and j + 1 < nblk:
                    # prefetch next block's one-hots and issue its phase A
                    # up front: its pool matmuls are DMA-paced, so they
                    # trickle into this block's PE drain gaps, and the
                    # pspool slot rotation leaves hnp an early slot
                    blocks.append(load_block(j + 1))
                    ga_next = phase_a(j + 1, blocks[j + 1][0])
                # GW1 = G @ W1^T + b1  (graph-major, bf16) — also the
                # stationary operand of the gather matmuls
                gw1p = psp.tile([GBLK, H], fp32, tag="pspool", name="gw1p")
                for ki in range(2):
                    nc.tensor.matmul(gw1p[:], g_fm[:, ki, :], w1t[ki][:],
                                     start=(ki == 0), stop=False,
                                     skip_group_check=True)
                nc.tensor.matmul(gw1p[:], ones_row[:], b1row[:],
                                 start=False, stop=True, skip_group_check=True)
                gw1 = gsb.tile([GBLK, H], bf16, tag="gw1")
                nc.scalar.copy(gw1[:], gw1p[:])

                pooled = psp.tile([GBLK, H], fp32, tag="pspool")
                gtall = gtp.tile([128, NT], bf16, tag="gtall")
                ng = NT // GB

                def pool_pair(pp):
                    """eg = ech * gate ; pooled += eg^T @ x for pair pp."""
                    t0 = pp * PGB
                    n = min(PGB, NT - t0)
                    xn = xn_stream(j, t0, n, xnstr, "xnstr")
                    # one batched DVE multiply for the whole pair: the gate
                    # column broadcasts over the 32 window columns
                    eg = epool.tile([128, PGB, WG], bf16, tag="eg")
                    nc.vector.tensor_tensor(
                        eg[:, 0:n, :], ej[:, t0 : t0 + n, :],
                        gtall[:, t0 : t0 + n].unsqueeze(-1).broadcast_to(
                            [128, n, WG]),
                        op=mybir.AluOpType.mult,
                    )
                    for c in range(n):
                        t = t0 + c
                        w = t // TW
                        nc.tensor.matmul(
                            pooled[w * WG : (w + 1) * WG, :], eg[:, c, :],
                            xn[:, c, :],
                            start=(t % TW == 0), stop=(t % TW == TW - 1),
                            skip_group_check=True, tile_position=(0, w * WG),
                        )

                # gate phase: group pairs with stationary-operand reuse;
                # pooling MMs interleave LAG pairs behind so the gate tail
                # (dot -> bounce -> sigmoid) never stalls PE
                LAG = 4
                for p0 in range(0, ng, 2):
                    gis = [gi for gi in (p0, p0 + 1) if gi < ng]
                    xts_l, h1t_l = [], []
                    for gi in gis:
                        xts = xtstr.tile([128, 2, GBW], fp8, tag="xtstr")
                        nc.sync.dma_start(
                            xts[:],
                            xt8_d[j, :, :, gi * GBW : (gi + 1) * GBW],
                        )
                        xts_l.append(xts)
                        h1t_l.append(
                            psp.tile([128, 2, GBW], fp32, tag="h1t",
                                     name="h1t")
                        )
                    for h in range(2):
                        for k in range(len(gis)):
                            nc.tensor.matmul(h1t_l[k][:, h, :], w1dr[h][:],
                                             xts_l[k][:], start=True,
                                             stop=False, perf_mode=DR,
                                             skip_group_check=True)
                        for k, gi in enumerate(gis):
                            nc.tensor.matmul(
                                h1t_l[k][:, h, :],
                                gw1[:, h * 128 : (h + 1) * 128],
                                etj[:, gi * GB : (gi + 1) * GB, :],
                                start=False, stop=True,
                                skip_group_check=True,
                            )
                    for k, gi in enumerate(gis):
                        rsb = trsh.tile([128, 2, GBW], fp8, tag="trsh")
                        if k == 0:
                            # relu on Scalar for group 0 of the pair
                            nc.scalar.activation(
                                rsb[:], h1t_l[k][:],
                                mybir.ActivationFunctionType.Relu)
                        else:
                            # ... and on Vector for group 1
                            nc.vector.tensor_scalar(
                                rsb[:], h1t_l[k][:], 0.0, None,
                                op0=mybir.AluOpType.max)
                        s_ps = psp.tile([16, GB, 128], fp32, tag="psrow")
                        nc.tensor.matmul(s_ps[:], w2dr[:], rsb[:],
                                         start=True, stop=True, perf_mode=DR)
                        if k == 0:
                            g_row = growp.tile([1, 2, GB, 128], bf16,
                                               tag="grow")
                        # fused sigmoid+bias evacuation of the dots (Scalar)
                        nc.scalar.activation(
                            g_row[:, k, :, :], s_ps[0:1, :, :],
                            mybir.ActivationFunctionType.Sigmoid, bias=b2s[:])
                    # one DRAM bounce per pair (HW DMA queues) transposes
                    # both raw dot rows into columns
                    npair = len(gis)
                    gscr = gscrp.tile([npair, GB * 128], bf16, tag="gscr",
                                      name="gscr")
                    nc.gpsimd.dma_start(gscr[:], g_row[:, :npair, :, :])
                    nc.gpsimd.dma_start(
                        gtall[:, p0 * GB : (p0 + npair) * GB],
                        gscr[:].rearrange("g (c n) -> n (g c)", n=128),
                    )
                    pp = p0 // 2
                    if pp >= LAG:
                        pool_pair(pp - LAG)
                npair_tot = (ng + 1) // 2
                for pp in range(max(0, npair_tot - LAG), npair_tot):
                    pool_pair(pp)
                ps_sb = gsb.tile([GBLK, H], fp32, tag="poolsb")
                nc.scalar.copy(ps_sb[:], pooled[:])
                pf = fm_copy(ps_sb[:], gsb, "poolfm", bf16)

                # ---- GRU cell (graph-major) ----
                gf, h_old = g_fm, g_gm

                def gru_mm(psum, wi, wh, bias_row, bcol0, bn):
                    mms = []
                    if wi is not None:
                        mms += [(pf[:, ki, :], wi[ki][:, bcol0 : bcol0 + bn])
                                for ki in range(2)]
                    if wh is not None:
                        mms += [(gf[:, ki, :], wh[ki][:, bcol0 : bcol0 + bn])
                                for ki in range(2)]
                    for i, (lhsT, rhs) in enumerate(mms):
                        nc.tensor.matmul(
                            psum[:], lhsT, rhs, start=(i == 0), stop=False,
                            skip_group_check=True,
                        )
                    nc.tensor.matmul(
                        psum[:], ones_row[:], bias_row, start=False, stop=True,
                        skip_group_check=True,
                    )

                # hn first: it only needs gf (ready at ts start), so its MMs
                # can overlap the pool drain
                hnp = psp.tile([GBLK, H], fp32, tag="pspool", name="hnp")
                gru_mm(hnp, None, whh, bhn[:], 2 * H, H)
                rp = psp.tile([GBLK, H], fp32, tag="pspool", name="rp")
                gru_mm(rp, wih, whh, brz[:, 0:H], 0, H)
                r = smallsb.tile([GBLK, H], fp32, tag="gru_r")
                nc.scalar.activation(r[:], rp[:], mybir.ActivationFunctionType.Sigmoid)
                t1 = smallsb.tile([GBLK, H], fp32, tag="gru_s1")
                nc.vector.tensor_mul(t1[:], r[:], hnp[:])
                zp = psp.tile([GBLK, H], fp32, tag="pspool", name="zp")
                gru_mm(zp, wih, whh, brz[:, H : 2 * H], H, H)
                z = smallsb.tile([GBLK, H], fp32, tag="gru_z")
                nc.scalar.activation(z[:], zp[:], mybir.ActivationFunctionType.Sigmoid)
                inp_ = psp.tile([GBLK, H], fp32, tag="pspool", name="inp_")
                gru_mm(inp_, wih, None, bin_[:], 2 * H, H)
                t2 = smallsb.tile([GBLK, H], fp32, tag="gru_s2")
                nc.vector.tensor_add(t2[:], t1[:], inp_[:])
                n = smallsb.tile([GBLK, H], fp32, tag="gru_n")
                nc.scalar.activation(n[:], t2[:], mybir.ActivationFunctionType.Tanh)
                t3 = smallsb.tile([GBLK, H], fp32, tag="gru_s1")
                nc.vector.tensor_sub(t3[:], h_old[:], n[:])
                t4 = smallsb.tile([GBLK, H], fp32, tag="gru_s2")
                nc.vector.tensor_mul(t4[:], z[:], t3[:])
                t5 = smallsb.tile([GBLK, H], fp32, tag="gru_s3")
                nc.vector.tensor_add(t5[:], n[:], t4[:])
                g_gm = gsb.tile([GBLK, H], fp32, tag="gsb")
                nc.scalar.activation(g_gm[:], t5[:],
                                     mybir.ActivationFunctionType.Relu)
                if ts < NUM_TIMESTEPS - 1:
                    g_fm = fm_copy(g_gm[:], gsb, "gfm", bf16)

            nc.sync.dma_start(out_d[j * GBLK : (j + 1) * GBLK, :], g_gm[:])

    nc.compile()
    return nc


def _prep_inputs(x, batch, counts, n_cores, nblk, TW=None):
    """Host-side shard + window-pad + layout. Returns (per_core, TW)."""
    import ml_dtypes

    G = n_cores * nblk * GBLK
    NWTOT = G // WG
    batch = np.asarray(batch).astype(np.int64)
    x = np.asarray(x, dtype=np.float32)

    win_edges = np.searchsorted(batch, np.arange(0, G + 1, WG))
    win_cnt = np.diff(win_edges)
    if TW is None:
        TW = int(np.ceil(win_cnt.max() / 128))
    NT = NWIN * TW
    NTP = NT * 128
    TWP = TW * 128  # padded nodes per window

    invc_all = (1.0 / np.maximum(counts, 1.0)).astype(np.float32)

    xb = x.astype(ml_dtypes.bfloat16)
    gar = np.arange(WG, dtype=np.int64)
    per_core = []
    for k in range(n_cores):
        xk = np.zeros((nblk * NTP, H), dtype=ml_dtypes.bfloat16)
        ech = np.zeros((nblk, 128, NT, WG), dtype=ml_dtypes.bfloat16)
        etoh = np.zeros((nblk, 128, NT, 128), dtype=ml_dtypes.float8_e4m3)
        for j in range(nblk):
            bi = k * nblk + j
            lb = np.full(NTP, -1, dtype=np.int64)
            for w in range(NWIN):
                W = bi * NWIN + w
                lo, hi = win_edges[W], win_edges[W + 1]
                cnt = hi - lo
                base = j * NTP + w * TWP
                xk[base : base + cnt] = xb[lo:hi]
                lb[w * TWP : w * TWP + cnt] = batch[lo:hi] - (bi * GBLK)
            lt = lb.reshape(NT, 128)
            # window-local one-hot scaled by 1/count: ech[p, t, c]
            wof = (np.arange(NT) // TW) * WG  # window col offset per tile
            m = lt[:, :, None] == (wof[:, None, None] + gar[None, None, :])
            vals = invc_all[bi * GBLK + np.clip(lt, 0, GBLK - 1)]
            ech[j] = (m * vals[:, :, None]).transpose(1, 0, 2).astype(
                ml_dtypes.bfloat16)
            # full-block one-hot transpose for the gather matmuls
            e = (lt[:, :, None] == np.arange(GBLK)[None, None, :])
            etoh[j] = e.transpose(2, 0, 1).astype(ml_dtypes.float8_e4m3)
        # pair-interleaved feature-major fp8: [p, ki, node]
        xkT8 = np.ascontiguousarray(
            xk.reshape(nblk, NTP, 2, 128).transpose(0, 3, 2, 1)
        ).astype(ml_dtypes.float8_e4m3)
        per_core.append({"xk": xk, "xkT8": xkT8, "ech": ech, "etoh": etoh})
    return per_core, TW


def _const_inputs(gate_w1, gate_b1, gate_w2, gate_b2, gru_w_ih, gru_w_hh,
                  gru_b_ih, gru_b_hh):
    import ml_dtypes

    f = np.float32
    bf = ml_dtypes.bfloat16
    f8 = ml_dtypes.float8_e4m3
    c = {}
    w1 = np.asarray(gate_w1, f)
    c["w1t"] = np.ascontiguousarray(w1.T.reshape(2, 128, H)).astype(bf)
    # DoubleRow stationary: w1dr[h, p, i, m] = W1[h*128+m, i*128+p]
    c["w1dr"] = np.ascontiguousarray(
        w1.T.reshape(2, 128, 2, 128).transpose(2, 1, 0, 3)).astype(f8)
    c["b1row"] = np.asarray(gate_b1, f).reshape(1, H).astype(bf)
    w2p = np.zeros((128, 2, 16), f)
    w2p[:, :, 0] = np.asarray(gate_w2, f).reshape(2, 128).T
    c["w2dr"] = w2p.astype(f8)
    c["b2s"] = np.asarray(gate_b2, f).reshape(1, 1)
    c["wih_t"] = np.ascontiguousarray(
        np.asarray(gru_w_ih, f).T).reshape(2, 128, 3 * H).astype(bf)
    c["whh_t"] = np.ascontiguousarray(
        np.asarray(gru_w_hh, f).T).reshape(2, 128, 3 * H).astype(bf)
    bih = np.asarray(gru_b_ih, f)
    bhh = np.asarray(gru_b_hh, f)
    c["bsum_rz"] = (bih[: 2 * H] + bhh[: 2 * H]).reshape(1, 2 * H).astype(bf)
    c["bihn"] = bih[2 * H :].reshape(1, H).astype(bf)
    c["bhhn"] = bhh[2 * H :].reshape(1, H).astype(bf)
    c["eye128"] = np.eye(128, dtype=f)
    return c


_CACHE = {}


def run(x, gate_w1, gate_b1, gate_w2, gate_b2, gru_w_ih, gru_w_hh, gru_b_ih,
        gru_b_hh, batch, num_graphs, n_cores=8, nblk=NBLK, trace=False,
        use_sim=False):
    from concourse.bass_utils import run_bass_kernel_spmd

    batch = np.asarray(batch).astype(np.int64)
    G = n_cores * nblk * GBLK
    counts = np.bincount(batch, minlength=G).astype(np.float32)
    per_core, TW = _prep_inputs(x, batch, counts, n_cores, nblk)
    consts = _const_inputs(gate_w1, gate_b1, gate_w2, gate_b2, gru_w_ih,
                           gru_w_hh, gru_b_ih, gru_b_hh)
    in_maps = [{**consts, **pc} for pc in per_core]

    key = (TW, nblk, n_cores)
    if key not in _CACHE:
        _CACHE[key] = _build_program(TW, nblk=nblk)
    nc = _CACHE[key]

    if use_sim:
        from concourse.bass_interp import CoreSim

        outs = []
        for k in range(n_cores):
            sim = CoreSim(nc)
            for name, arr in in_maps[k].items():
                sim.tensor(name)[:] = arr
            sim.simulate()
            outs.append(np.array(sim.tensor("out")))
        return np.concatenate(outs, axis=0), None

    res = run_bass_kernel_spmd(nc, in_maps, core_ids=list(range(n_cores)),
                               trace=trace)
    out = np.concatenate([res.results[k]["out"] for k in range(n_cores)], axis=0)
    return out, res


def kernel(**inputs):
    out, _ = run(**inputs)
    return out


# revision 31
# speedup vs baseline: 1.1858x; 1.0295x over previous
"""Trainium2 Bass kernel for AttentiveGraphPooling (gnn_message_passing).

Strategy: shard the 4096 graphs across 8 cores (512 graphs each). batch is
sorted, so each core owns a contiguous node range covering whole graphs ->
pooling / gather / GRU are all core-local, no collectives needed.

Per core, graphs go in 4 blocks of 128, each block in 4 windows of 32
graphs. The host pads each window's nodes to a fixed TW node-tiles so the
program is uniform across cores. Pooling uses a [128, 32] gate-weighted
one-hot stationary per node tile (host-prescaled by 1/count so the PSUM
accumulation directly yields the mean); each window accumulates into its
own 32-partition strip of the [128, 256] pooled PSUM tile.

The gate MLP runs feature-major so every matmul has a *block-constant*
stationary operand:
  h1T[f, n] = W1 @ (x + g_b)^T = (fp8 DoubleRow: W1-pairs stationary, x^T
  pair-interleaved moving, N=512 per 4-tile group) + (gather: gw1
  graph-major half-columns stationary, E^T fp8 moving).
relu(h1T) -> SBUF fp8 (split Scalar/Vector by group parity), then the
gate dot w2 . relu(h1T) runs on the PE (w2 half-columns stationary,
N=512) into a [1, 512] PSUM row; a fused sigmoid+bias activation
(Scalar) evacuates it to SBUF; a DRAM bounce (gpsimd-executed DMA)
transposes the gate rows into per-tile columns. Weighted pooling is
eg^T @ x on the PE with the 32-wide stationary (eg batched per 8-tile
chunk on DVE via a broadcast multiply). The GRU runs per graph-block
(hn-first so its matmuls overlap the pool drain) with bf16 K=1 bias
matmuls; block j+1's phase A is issued at block j's ts0 start so its
DMA-paced pool matmuls fill PE drain gaps.
"""

import os
import sys

import numpy as np

sys.path.insert(0, "/opt/trn_rl_repo")

H = 256
NBLK = 4  # graph blocks per core
GBLK = 128  # graphs per block
NWIN = 4  # windows per block (32 graphs each)
WG = 32  # graphs per window
NUM_TIMESTEPS = 2
GB = 4  # node tiles per gate-pipeline group
PGB = 8  # node tiles per streamed node-major x group


def _build_program(TW, nblk=NBLK):
    """Build the single-core SPMD Bass program. TW = node tiles per window."""
    from contextlib import ExitStack

    import concourse.bass as bass
    import concourse.tile as tile
    from concourse import bacc, mybir

    fp32 = mybir.dt.float32
    bf16 = mybir.dt.bfloat16
    fp8 = mybir.dt.float8e4
    DR = mybir.MatmulPerfMode.DoubleRow

    NT = NWIN * TW  # node tiles per block
    NTP = NT * 128  # padded nodes per block
    GBW = GB * 128

    nc = bacc.Bacc("TRN2", target_bir_lowering=False, debug=False)

    # ---- DRAM parameters (per-core inputs) ----
    x_d = nc.dram_tensor("xk", [nblk * NTP, H], bf16, kind="ExternalInput")
    xt8_d = nc.dram_tensor("xkT8", [nblk, 128, 2, NTP], fp8, kind="ExternalInput")
    e_d = nc.dram_tensor("ech", [nblk, 128, NT, WG], bf16, kind="ExternalInput")
    etj_d = nc.dram_tensor("etoh", [nblk, 128, NT, 128], fp8, kind="ExternalInput")
    w1t_d = nc.dram_tensor("w1t", [2, 128, H], bf16, kind="ExternalInput")
    w1dr_d = nc.dram_tensor("w1dr", [2, 128, 2, 128], fp8, kind="ExternalInput")
    b1r_d = nc.dram_tensor("b1row", [1, H], bf16, kind="ExternalInput")
    w2c_d = nc.dram_tensor("w2dr", [128, 2, 16], fp8, kind="ExternalInput")
    b2c_d = nc.dram_tensor("b2s", [1, 1], fp32, kind="ExternalInput")
    wih_d = nc.dram_tensor("wih_t", [2, 128, 3 * H], bf16, kind="ExternalInput")
    whh_d = nc.dram_tensor("whh_t", [2, 128, 3 * H], bf16, kind="ExternalInput")
    brz_d = nc.dram_tensor("bsum_rz", [1, 2 * H], bf16, kind="ExternalInput")
    bin_d = nc.dram_tensor("bihn", [1, H], bf16, kind="ExternalInput")
    bhn_d = nc.dram_tensor("bhhn", [1, H], bf16, kind="ExternalInput")
    eye_d = nc.dram_tensor("eye128", [128, 128], fp32, kind="ExternalInput")
    out_d = nc.dram_tensor("out", [nblk * GBLK, H], fp32, kind="ExternalOutput")

    with tile.TileContext(nc) as tc, ExitStack() as ctx:
        ep = ctx.enter_context  # shorthand

        const = ep(tc.tile_pool(name="const", bufs=1))
        eres = ep(tc.tile_pool(name="eres", bufs=2))
        etres = ep(tc.tile_pool(name="etres", bufs=2))
        xtstr = ep(tc.tile_pool(name="xtstr", bufs=4))
        xnstr = ep(tc.tile_pool(name="xnstr", bufs=6))
        xastr = ep(tc.tile_pool(name="xastr", bufs=4))
        epool = ep(tc.tile_pool(name="egpool", bufs=8))
        trsh = ep(tc.tile_pool(name="trsh", bufs=3))
        growp = ep(tc.tile_pool(name="grow", bufs=3))
        gtp = ep(tc.tile_pool(name="gtp", bufs=2))
        gsb = ep(tc.tile_pool(name="gsb", bufs=3))
        smallsb = ep(tc.tile_pool(name="smallsb", bufs=2))

        gscrp = ep(tc.tile_pool(name="gscr", bufs=4, space="DRAM"))

        psp = ep(tc.tile_pool(name="psp", bufs=2, space="PSUM"))

        # ---- load constants ----
        def cload(shape, src, tag, dt=fp32):
            t = const.tile(shape, dt, tag=tag)
            nc.sync.dma_start(t[:], src)
            return t

        eye = cload([128, 128], eye_d[:], "c_eye")
        w1t = [cload([128, H], w1t_d[k], f"c_w1t{k}", bf16) for k in range(2)]
        w1dr = [cload([128, 2, 128], w1dr_d[h], f"c_w1dr{h}", fp8)
                for h in range(2)]
        b1row = cload([1, H], b1r_d[:], "c_b1r", bf16)
        w2dr = cload([128, 2, 16], w2c_d[:], "c_w2dr", fp8)
        b2s = cload([1, 1], b2c_d[:], "c_b2s")
        wih = [cload([128, 3 * H], wih_d[k], f"c_wih{k}", bf16) for k in range(2)]
        whh = [cload([128, 3 * H], whh_d[k], f"c_whh{k}", bf16) for k in range(2)]
        brz = cload([1, 2 * H], brz_d[:], "c_brz", bf16)
        bin_ = cload([1, H], bin_d[:], "c_bin", bf16)
        bhn = cload([1, H], bhn_d[:], "c_bhn", bf16)
        ones_row = const.tile([1, 128], bf16)
        nc.vector.memset(ones_row[:], 1.0)

        def fm_copy(g_ap, pool, tag, dt):
            """(128,256) graph-major -> feature-major (128,2,128) via PE."""
            gf = pool.tile([128, 2, GBLK], dt, tag=tag)
            for ki in range(2):
                tp = psp.tile([128, 128], fp32, tag="psrow", name="tp")
                nc.tensor.matmul(tp[:], g_ap[:, ki * 128 : (ki + 1) * 128], eye[:],
                                 is_transpose=True, start=True, stop=True)
                nc.scalar.copy(gf[:, ki, :], tp[:])
            return gf

        blocks = []

        def load_block(j):
            """DMA one-hot chunks for block j."""
            ej = eres.tile([128, NT, WG], bf16, tag="eres", name="ej")
            nc.sync.dma_start(ej[:], e_d[j])
            etj = etres.tile([128, NT, 128], fp8, tag="etres", name="etj")
            nc.sync.dma_start(etj[:], etj_d[j])
            return ej, etj

        def xn_stream(j, t0, n, pool, tag):
            """Stream node-major x tiles [t0, t0+n) of block j."""
            xn = pool.tile([128, PGB, H], bf16, tag=tag, name="xn")
            base = j * NTP + t0 * 128
            nc.sync.dma_start(
                xn[:, 0:n, :], x_d[base : base + n * 128, :].rearrange(
                    "(c p) h -> p c h", p=128
                )
            )
            return xn

        blocks.append(load_block(0))
        for j in range(nblk):
            ej, etj = blocks[j]

            def phase_a(j, ej):
                """Initial mean pool for block j -> (g_gm, g_fm)."""
                pooled = psp.tile([GBLK, H], fp32, tag="pspool")
                for t0 in range(0, NT, PGB):
                    na = min(PGB, NT - t0)
                    xn = xn_stream(j, t0, na, xastr, "xastr")
                    for c in range(na):
                        t = t0 + c
                        w = t // TW
                        nc.tensor.matmul(
                            pooled[w * WG : (w + 1) * WG, :],
                            ej[:, t, :], xn[:, c, :],
                            start=(t % TW == 0), stop=(t % TW == TW - 1),
                            skip_group_check=True, tile_position=(0, w * WG),
                        )
                g_gm = gsb.tile([GBLK, H], fp32, tag="gsb")
                nc.scalar.copy(g_gm[:], pooled[:])
                return g_gm, fm_copy(g_gm[:], gsb, "gfm", bf16)

            if j == 0:
                g_gm, g_fm = phase_a(0, ej)
            else:
                g_gm, g_fm = ga_next

            # ---- timesteps ----
            for ts in range(NUM_TIMESTEPS):
                if ts == 0 and j + 1 < nblk:
                    # prefetch next block's one-hots and issue its phase A
                    # up front: its pool matmuls are DMA-paced, so they
                    # trickle into this block's PE drain gaps, and the
                    # pspool slot rotation leaves hnp an early slot
                    blocks.append(load_block(j + 1))
                    ga_next = phase_a(j + 1, blocks[j + 1][0])
                # GW1 = G @ W1^T + b1  (graph-major, bf16) — also the
                # stationary operand of the gather matmuls
                gw1p = psp.tile([GBLK, H], fp32, tag="pspool", name="gw1p")
                for ki in range(2):
                    nc.tensor.matmul(gw1p[:], g_fm[:, ki, :], w1t[ki][:],
                                     start=(ki == 0), stop=False,
                                     skip_group_check=True)
                nc.tensor.matmul(gw1p[:], ones_row[:], b1row[:],
                                 start=False, stop=True, skip_group_check=True)
                gw1 = gsb.tile([GBLK, H], bf16, tag="gw1")
                nc.scalar.copy(gw1[:], gw1p[:])

                pooled = psp.tile([GBLK, H], fp32, tag="pspool")
                gtall = gtp.tile([128, NT], fp32, tag="gtall")
                ng = NT // GB

                def pool_pair(pp):
                    """eg = ech * gate ; pooled += eg^T @ x for pair pp."""
                    t0 = pp * PGB
                    n = min(PGB, NT - t0)
                    xn = xn_stream(j, t0, n, xnstr, "xnstr")
                    # one batched DVE multiply for the whole pair: the gate
                    # column broadcasts over the 32 window columns
                    eg = epool.tile([128, PGB, WG], bf16, tag="eg")
                    nc.vector.tensor_tensor(
                        eg[:, 0:n, :], ej[:, t0 : t0 + n, :],
                        gtall[:, t0 : t0 + n].unsqueeze(-1).broadcast_to(
                            [128, n, WG]),
                        op=mybir.AluOpType.mult,
                    )
                    for c in range(n):
                        t = t0 + c
                        w = t // TW
                        nc.tensor.matmul(
                            pooled[w * WG : (w + 1) * WG, :], eg[:, c, :],
                            xn[:, c, :],
                            start=(t % TW == 0), stop=(t % TW == TW - 1),
                            skip_group_check=True, tile_position=(0, w * WG),
                        )

                # gate phase: group pairs with stationary-operand reuse;
                # pooling MMs interleave LAG pairs behind so the gate tail
                # (dot -> bounce -> sigmoid) never stalls PE
                LAG = 3
                for p0 in range(0, ng, 2):
                    gis = [gi for gi in (p0, p0 + 1) if gi < ng]
                    xts_l, h1t_l = [], []
                    for gi in gis:
                        xts = xtstr.tile([128, 2, GBW], fp8, tag="xtstr")
                        nc.sync.dma_start(
                            xts[:],
                            xt8_d[j, :, :, gi * GBW : (gi + 1) * GBW],
                        )
                        xts_l.append(xts)
                        h1t_l.append(
                            psp.tile([128, 2, GBW], fp32, tag="h1t",
                                     name="h1t")
                        )
                    for h in range(2):
                        for k in range(len(gis)):
                            nc.tensor.matmul(h1t_l[k][:, h, :], w1dr[h][:],
                                             xts_l[k][:], start=True,
                                             stop=False, perf_mode=DR,
                                             skip_group_check=True)
                        for k, gi in enumerate(gis):
                            nc.tensor.matmul(
                                h1t_l[k][:, h, :],
                                gw1[:, h * 128 : (h + 1) * 128],
                                etj[:, gi * GB : (gi + 1) * GB, :],
                                start=False, stop=True,
                                skip_group_check=True,
                            )
                    for k, gi in enumerate(gis):
                        rsb = trsh.tile([128, 2, GBW], fp8, tag="trsh")
                        if k == 0:
                            # relu on Scalar for group 0 of the pair
                            nc.scalar.activation(
                                rsb[:], h1t_l[k][:],
                                mybir.ActivationFunctionType.Relu)
                        else:
                            # ... and on Vector for group 1
                            nc.vector.tensor_scalar(
                                rsb[:], h1t_l[k][:], 0.0, None,
                                op0=mybir.AluOpType.max)
                        s_ps = psp.tile([16, GB, 128], fp32, tag="psrow")
                        nc.tensor.matmul(s_ps[:], w2dr[:], rsb[:],
                                         start=True, stop=True, perf_mode=DR)
                        if k == 0:
                            g_row = growp.tile([1, 2, GB, 128], fp32,
                                               tag="grow")
                        # fused sigmoid+bias evacuation of the dots (Scalar)
                        nc.scalar.activation(
                            g_row[:, k, :, :], s_ps[0:1, :, :],
                            mybir.ActivationFunctionType.Sigmoid, bias=b2s[:])
                    # one DRAM bounce per pair (HW DMA queues) transposes
                    # both raw dot rows into columns
                    npair = len(gis)
                    gscr = gscrp.tile([npair, GB * 128], fp32, tag="gscr",
                                      name="gscr")
                    nc.gpsimd.dma_start(gscr[:], g_row[:, :npair, :, :])
                    nc.gpsimd.dma_start(
                        gtall[:, p0 * GB : (p0 + npair) * GB],
                        gscr[:].rearrange("g (c n) -> n (g c)", n=128),
                    )
                    pp = p0 // 2
                    if pp >= LAG:
                        pool_pair(pp - LAG)
                npair_tot = (ng + 1) // 2
                for pp in range(max(0, npair_tot - LAG), npair_tot):
                    pool_pair(pp)
                ps_sb = gsb.tile([GBLK, H], fp32, tag="poolsb")
                nc.scalar.copy(ps_sb[:], pooled[:])
                pf = fm_copy(ps_sb[:], gsb, "poolfm", bf16)

                # ---- GRU cell (graph-major) ----
                gf, h_old = g_fm, g_gm

                def gru_mm(psum, wi, wh, bias_row, bcol0, bn):
                    mms = []
                    if wi is not None:
                        mms += [(pf[:, ki, :], wi[ki][:, bcol0 : bcol0 + bn])
                                for ki in range(2)]
                    if wh is not None:
                        mms += [(gf[:, ki, :], wh[ki][:, bcol0 : bcol0 + bn])
                                for ki in range(2)]
                    for i, (lhsT, rhs) in enumerate(mms):
                        nc.tensor.matmul(
                            psum[:], lhsT, rhs, start=(i == 0), stop=False,
                            skip_group_check=True,
                        )
                    nc.tensor.matmul(
                        psum[:], ones_row[:], bias_row, start=False, stop=True,
                        skip_group_check=True,
                    )

                # hn first: it only needs gf (ready at ts start), so its MMs
                # can overlap the pool drain
                hnp = psp.tile([GBLK, H], fp32, tag="pspool", name="hnp")
                gru_mm(hnp, None, whh, bhn[:], 2 * H, H)
                rp = psp.tile([GBLK, H], fp32, tag="pspool", name="rp")
                gru_mm(rp, wih, whh, brz[:, 0:H], 0, H)
                r = smallsb.tile([GBLK, H], fp32, tag="gru_r")
                nc.scalar.activation(r[:], rp[:], mybir.ActivationFunctionType.Sigmoid)
                t1 = smallsb.tile([GBLK, H], fp32, tag="gru_s1")
                nc.vector.tensor_mul(t1[:], r[:], hnp[:])
                zp = psp.tile([GBLK, H], fp32, tag="pspool", name="zp")
                gru_mm(zp, wih, whh, brz[:, H : 2 * H], H, H)
                z = smallsb.tile([GBLK, H], fp32, tag="gru_z")
                nc.scalar.activation(z[:], zp[:], mybir.ActivationFunctionType.Sigmoid)
                inp_ = psp.tile([GBLK, H], fp32, tag="pspool", name="inp_")
                gru_mm(inp_, wih, None, bin_[:], 2 * H, H)
                t2 = smallsb.tile([GBLK, H], fp32, tag="gru_s2")
                nc.vector.tensor_add(t2[:], t1[:], inp_[:])
                n = smallsb.tile([GBLK, H], fp32, tag="gru_n")
                nc.scalar.activation(n[:], t2[:], mybir.ActivationFunctionType.Tanh)
                t3 = smallsb.tile([GBLK, H], fp32, tag="gru_s1")
                nc.vector.tensor_sub(t3[:], h_old[:], n[:])
                t4 = smallsb.tile([GBLK, H], fp32, tag="gru_s2")
                nc.vector.tensor_mul(t4[:], z[:], t3[:])
                t5 = smallsb.tile([GBLK, H], fp32, tag="gru_s3")
                nc.vector.tensor_add(t5[:], n[:], t4[:])
                g_gm = gsb.tile([GBLK, H], fp32, tag="gsb")
                nc.scalar.activation(g_gm[:], t5[:],
                                     mybir.ActivationFunctionType.Relu)
                if ts < NUM_TIMESTEPS - 1:
                    g_fm = fm_copy(g_gm[:], gsb, "gfm", bf16)

            nc.sync.dma_start(out_d[j * GBLK : (j + 1) * GBLK, :], g_gm[:])

    nc.compile()
    return nc


def _prep_inputs(x, batch, counts, n_cores, nblk, TW=None):
    """Host-side shard + window-pad + layout. Returns (per_core, TW)."""
    import ml_dtypes

    G = n_cores * nblk * GBLK
    NWTOT = G // WG
    batch = np.asarray(batch).astype(np.int64)
    x = np.asarray(x, dtype=np.float32)

    win_edges = np.searchsorted(batch, np.arange(0, G + 1, WG))
    win_cnt = np.diff(win_edges)
    if TW is None:
        TW = int(np.ceil(win_cnt.max() / 128))
    NT = NWIN * TW
    NTP = NT * 128
    TWP = TW * 128  # padded nodes per window

    invc_all = (1.0 / np.maximum(counts, 1.0)).astype(np.float32)

    xb = x.astype(ml_dtypes.bfloat16)
    gar = np.arange(WG, dtype=np.int64)
    per_core = []
    for k in range(n_cores):
        xk = np.zeros((nblk * NTP, H), dtype=ml_dtypes.bfloat16)
        ech = np.zeros((nblk, 128, NT, WG), dtype=ml_dtypes.bfloat16)
        etoh = np.zeros((nblk, 128, NT, 128), dtype=ml_dtypes.float8_e4m3)
        for j in range(nblk):
            bi = k * nblk + j
            lb = np.full(NTP, -1, dtype=np.int64)
            for w in range(NWIN):
                W = bi * NWIN + w
                lo, hi = win_edges[W], win_edges[W + 1]
                cnt = hi - lo
                base = j * NTP + w * TWP
                xk[base : base + cnt] = xb[lo:hi]
                lb[w * TWP : w * TWP + cnt] = batch[lo:hi] - (bi * GBLK)
            lt = lb.reshape(NT, 128)
            # window-local one-hot scaled by 1/count: ech[p, t, c]
            wof = (np.arange(NT) // TW) * WG  # window col offset per tile
            m = lt[:, :, None] == (wof[:, None, None] + gar[None, None, :])
            vals = invc_all[bi * GBLK + np.clip(lt, 0, GBLK - 1)]
            ech[j] = (m * vals[:, :, None]).transpose(1, 0, 2).astype(
                ml_dtypes.bfloat16)
            # full-block one-hot transpose for the gather matmuls
            e = (lt[:, :, None] == np.arange(GBLK)[None, None, :])
            etoh[j] = e.transpose(2, 0, 1).astype(ml_dtypes.float8_e4m3)
        # pair-interleaved feature-major fp8: [p, ki, node]
        xkT8 = np.ascontiguousarray(
            xk.reshape(nblk, NTP, 2, 128).transpose(0, 3, 2, 1)
        ).astype(ml_dtypes.float8_e4m3)
        per_core.append({"xk": xk, "xkT8": xkT8, "ech": ech, "etoh": etoh})
    return per_core, TW


def _const_inputs(gate_w1, gate_b1, gate_w2, gate_b2, gru_w_ih, gru_w_hh,
                  gru_b_ih, gru_b_hh):
    import ml_dtypes

    f = np.float32
    bf = ml_dtypes.bfloat16
    f8 = ml_dtypes.float8_e4m3
    c = {}
    w1 = np.asarray(gate_w1, f)
    c["w1t"] = np.ascontiguousarray(w1.T.reshape(2, 128, H)).astype(bf)
    # DoubleRow stationary: w1dr[h, p, i, m] = W1[h*128+m, i*128+p]
    c["w1dr"] = np.ascontiguousarray(
        w1.T.reshape(2, 128, 2, 128).transpose(2, 1, 0, 3)).astype(f8)
    c["b1row"] = np.asarray(gate_b1, f).reshape(1, H).astype(bf)
    w2p = np.zeros((128, 2, 16), f)
    w2p[:, :, 0] = np.asarray(gate_w2, f).reshape(2, 128).T
    c["w2dr"] = w2p.astype(f8)
    c["b2s"] = np.asarray(gate_b2, f).reshape(1, 1)
    c["wih_t"] = np.ascontiguousarray(
        np.asarray(gru_w_ih, f).T).reshape(2, 128, 3 * H).astype(bf)
    c["whh_t"] = np.ascontiguousarray(
        np.asarray(gru_w_hh, f).T).reshape(2, 128, 3 * H).astype(bf)
    bih = np.asarray(gru_b_ih, f)
    bhh = np.asarray(gru_b_hh, f)
    c["bsum_rz"] = (bih[: 2 * H] + bhh[: 2 * H]).reshape(1, 2 * H).astype(bf)
    c["bihn"] = bih[2 * H :].reshape(1, H).astype(bf)
    c["bhhn"] = bhh[2 * H :].reshape(1, H).astype(bf)
    c["eye128"] = np.eye(128, dtype=f)
    return c


_CACHE = {}


def run(x, gate_w1, gate_b1, gate_w2, gate_b2, gru_w_ih, gru_w_hh, gru_b_ih,
        gru_b_hh, batch, num_graphs, n_cores=8, nblk=NBLK, trace=False,
        use_sim=False):
    from concourse.bass_utils import run_bass_kernel_spmd

    batch = np.asarray(batch).astype(np.int64)
    G = n_cores * nblk * GBLK
    counts = np.bincount(batch, minlength=G).astype(np.float32)
    per_core, TW = _prep_inputs(x, batch, counts, n_cores, nblk)
    consts = _const_inputs(gate_w1, gate_b1, gate_w2, gate_b2, gru_w_ih,
                           gru_w_hh, gru_b_ih, gru_b_hh)
    in_maps = [{**consts, **pc} for pc in per_core]

    key = (TW, nblk, n_cores)
    if key not in _CACHE:
        _CACHE[key] = _build_program(TW, nblk=nblk)
    nc = _CACHE[key]

    if use_sim:
        from concourse.bass_interp import CoreSim

        outs = []
        for k in range(n_cores):
            sim = CoreSim(nc)
            for name, arr in in_maps[k].items():
                sim.tensor(name)[:] = arr
            sim.simulate()
            outs.append(np.array(sim.tensor("out")))
        return np.concatenate(outs, axis=0), None

    res = run_bass_kernel_spmd(nc, in_maps, core_ids=list(range(n_cores)),
                               trace=trace)
    out = np.concatenate([res.results[k]["out"] for k in range(n_cores)], axis=0)
    return out, res


def kernel(**inputs):
    out, _ = run(**inputs)
    return out


# revision 32
# speedup vs baseline: 1.2569x; 1.0599x over previous
"""Trainium2 Bass kernel for AttentiveGraphPooling (gnn_message_passing).

Strategy: shard the 4096 graphs across 8 cores (512 graphs each). batch is
sorted, so each core owns a contiguous node range covering whole graphs ->
pooling / gather / GRU are all core-local, no collectives needed.

Per core, graphs go in 4 blocks of 128, each block in 4 windows of 32
graphs. The host pads each window's nodes to a fixed TW node-tiles so the
program is uniform across cores. Pooling uses a [128, 32] gate-weighted
one-hot stationary per node tile (host-prescaled by 1/count so the PSUM
accumulation directly yields the mean); each window accumulates into its
own 32-partition strip of the [128, 256] pooled PSUM tile.

The gate MLP runs feature-major so every matmul has a *block-constant*
stationary operand:
  h1T[f, n] = W1 @ (x + g_b)^T = (fp8 DoubleRow: W1-pairs stationary, x^T
  pair-interleaved moving, N=512 per 4-tile group) + (gather: gw1
  graph-major half-columns stationary, E^T fp8 moving).
relu(h1T) -> SBUF fp8 (split Scalar/Vector by group parity), then the
gate dot w2 . relu(h1T) runs on the PE (w2 half-columns stationary,
N=512) into a [1, 512] PSUM row; a fused sigmoid+bias activation
(Scalar) evacuates it to SBUF; a DRAM bounce (gpsimd-executed DMA)
transposes the gate rows into per-tile columns. Weighted pooling is
eg^T @ x on the PE with the 32-wide stationary (eg batched per 8-tile
chunk on DVE via a broadcast multiply). The GRU runs per graph-block
(hn-first so its matmuls overlap the pool drain) with bf16 K=1 bias
matmuls; block j+1's phase A is issued at block j's ts0 start so its
DMA-paced pool matmuls fill PE drain gaps.
"""

import os
import sys

import numpy as np

sys.path.insert(0, "/opt/trn_rl_repo")

H = 256
NBLK = 4  # graph blocks per core
GBLK = 128  # graphs per block
NWIN = 4  # windows per block (32 graphs each)
WG = 32  # graphs per window
NUM_TIMESTEPS = 2
GB = 4  # node tiles per gate-pipeline group
PGB = 8  # node tiles per streamed node-major x group


def _build_program(TW, nblk=NBLK):
    """Build the single-core SPMD Bass program. TW = node tiles per window."""
    from contextlib import ExitStack

    import concourse.bass as bass
    import concourse.tile as tile
    from concourse import bacc, mybir

    fp32 = mybir.dt.float32
    bf16 = mybir.dt.bfloat16
    fp8 = mybir.dt.float8e4
    DR = mybir.MatmulPerfMode.DoubleRow

    NT = NWIN * TW  # node tiles per block
    NTP = NT * 128  # padded nodes per block
    GBW = GB * 128

    nc = bacc.Bacc("TRN2", target_bir_lowering=False, debug=False)

    # ---- DRAM parameters (per-core inputs) ----
    x_d = nc.dram_tensor("xk", [nblk * NTP, H], bf16, kind="ExternalInput")
    xt8_d = nc.dram_tensor("xkT8", [nblk, 128, 2, NTP], fp8, kind="ExternalInput")
    e_d = nc.dram_tensor("ech", [nblk, 128, NT, WG], bf16, kind="ExternalInput")
    etj_d = nc.dram_tensor("etoh", [nblk, 128, NT, 128], fp8, kind="ExternalInput")
    w1t_d = nc.dram_tensor("w1t", [2, 128, H], bf16, kind="ExternalInput")
    w1dr_d = nc.dram_tensor("w1dr", [2, 128, 2, 128], fp8, kind="ExternalInput")
    b1r_d = nc.dram_tensor("b1row", [1, H], bf16, kind="ExternalInput")
    w2c_d = nc.dram_tensor("w2dr", [128, 2, 16], fp8, kind="ExternalInput")
    b2c_d = nc.dram_tensor("b2s", [1, 1], fp32, kind="ExternalInput")
    wih_d = nc.dram_tensor("wih_t", [2, 128, 3 * H], bf16, kind="ExternalInput")
    whh_d = nc.dram_tensor("whh_t", [2, 128, 3 * H], bf16, kind="ExternalInput")
    brz_d = nc.dram_tensor("bsum_rz", [1, 2 * H], bf16, kind="ExternalInput")
    bin_d = nc.dram_tensor("bihn", [1, H], bf16, kind="ExternalInput")
    bhn_d = nc.dram_tensor("bhhn", [1, H], bf16, kind="ExternalInput")
    eye_d = nc.dram_tensor("eye128", [128, 128], fp32, kind="ExternalInput")
    out_d = nc.dram_tensor("out", [nblk * GBLK, H], fp32, kind="ExternalOutput")

    with tile.TileContext(nc) as tc, ExitStack() as ctx:
        ep = ctx.enter_context  # shorthand

        const = ep(tc.tile_pool(name="const", bufs=1))
        eres = ep(tc.tile_pool(name="eres", bufs=2))
        etres = ep(tc.tile_pool(name="etres", bufs=2))
        xtstr = ep(tc.tile_pool(name="xtstr", bufs=4))
        xnstr = ep(tc.tile_pool(name="xnstr", bufs=10))
        xastr = ep(tc.tile_pool(name="xastr", bufs=8))
        epool = ep(tc.tile_pool(name="egpool", bufs=10))
        trsh = ep(tc.tile_pool(name="trsh", bufs=3))
        growp = ep(tc.tile_pool(name="grow", bufs=3))
        gtp = ep(tc.tile_pool(name="gtp", bufs=2))
        gsb = ep(tc.tile_pool(name="gsb", bufs=3))
        smallsb = ep(tc.tile_pool(name="smallsb", bufs=2))

        gscrp = ep(tc.tile_pool(name="gscr", bufs=4, space="DRAM"))

        psp = ep(tc.tile_pool(name="psp", bufs=2, space="PSUM"))

        # ---- load constants ----
        def cload(shape, src, tag, dt=fp32):
            t = const.tile(shape, dt, tag=tag)
            nc.sync.dma_start(t[:], src)
            return t

        eye = cload([128, 128], eye_d[:], "c_eye")
        w1t = [cload([128, H], w1t_d[k], f"c_w1t{k}", bf16) for k in range(2)]
        w1dr = [cload([128, 2, 128], w1dr_d[h], f"c_w1dr{h}", fp8)
                for h in range(2)]
        b1row = cload([1, H], b1r_d[:], "c_b1r", bf16)
        w2dr = cload([128, 2, 16], w2c_d[:], "c_w2dr", fp8)
        b2s = cload([1, 1], b2c_d[:], "c_b2s")
        wih = [cload([128, 3 * H], wih_d[k], f"c_wih{k}", bf16) for k in range(2)]
        whh = [cload([128, 3 * H], whh_d[k], f"c_whh{k}", bf16) for k in range(2)]
        brz = cload([1, 2 * H], brz_d[:], "c_brz", bf16)
        bin_ = cload([1, H], bin_d[:], "c_bin", bf16)
        bhn = cload([1, H], bhn_d[:], "c_bhn", bf16)
        ones_row = const.tile([1, 128], bf16)
        nc.vector.memset(ones_row[:], 1.0)

        def fm_copy(g_ap, pool, tag, dt):
            """(128,256) graph-major -> feature-major (128,2,128) via PE."""
            gf = pool.tile([128, 2, GBLK], dt, tag=tag)
            for ki in range(2):
                tp = psp.tile([128, 128], fp32, tag="psrow", name="tp")
                nc.tensor.matmul(tp[:], g_ap[:, ki * 128 : (ki + 1) * 128], eye[:],
                                 is_transpose=True, start=True, stop=True)
                nc.scalar.copy(gf[:, ki, :], tp[:])
            return gf

        blocks = []

        def load_block(j):
            """DMA one-hot chunks for block j."""
            ej = eres.tile([128, NT, WG], bf16, tag="eres", name="ej")
            nc.sync.dma_start(ej[:], e_d[j])
            etj = etres.tile([128, NT, 128], fp8, tag="etres", name="etj")
            nc.sync.dma_start(etj[:], etj_d[j])
            return ej, etj

        def xn_stream(j, t0, n, pool, tag):
            """Stream node-major x tiles [t0, t0+n) of block j."""
            xn = pool.tile([128, PGB, H], bf16, tag=tag, name="xn")
            base = j * NTP + t0 * 128
            nc.sync.dma_start(
                xn[:, 0:n, :], x_d[base : base + n * 128, :].rearrange(
                    "(c p) h -> p c h", p=128
                )
            )
            return xn

        blocks.append(load_block(0))
        for j in range(nblk):
            ej, etj = blocks[j]

            def phase_a(j, ej):
                """Initial mean pool for block j -> (g_gm, g_fm)."""
                pooled = psp.tile([GBLK, H], fp32, tag="pspool")
                for t0 in range(0, NT, PGB):
                    na = min(PGB, NT - t0)
                    xn = xn_stream(j, t0, na, xastr, "xastr")
                    for c in range(na):
                        t = t0 + c
                        w = t // TW
                        nc.tensor.matmul(
                            pooled[w * WG : (w + 1) * WG, :],
                            ej[:, t, :], xn[:, c, :],
                            start=(t % TW == 0), stop=(t % TW == TW - 1),
                            skip_group_check=True, tile_position=(0, w * WG),
                        )
                g_gm = gsb.tile([GBLK, H], fp32, tag="gsb")
                nc.scalar.copy(g_gm[:], pooled[:])
                return g_gm, fm_copy(g_gm[:], gsb, "gfm", bf16)

            if j == 0:
                g_gm, g_fm = phase_a(0, ej)
            else:
                g_gm, g_fm = ga_next

            # ---- timesteps ----
            for ts in range(NUM_TIMESTEPS):
                if ts == 0 and j + 1 < nblk:
                    # prefetch next block's one-hots and issue its phase A
                    # up front: its pool matmuls are DMA-paced, so they
                    # trickle into this block's PE drain gaps, and the
                    # pspool slot rotation leaves hnp an early slot
                    blocks.append(load_block(j + 1))
                    ga_next = phase_a(j + 1, blocks[j + 1][0])
                # GW1 = G @ W1^T + b1  (graph-major, bf16) — also the
                # stationary operand of the gather matmuls
                gw1p = psp.tile([GBLK, H], fp32, tag="pspool", name="gw1p")
                for ki in range(2):
                    nc.tensor.matmul(gw1p[:], g_fm[:, ki, :], w1t[ki][:],
                                     start=(ki == 0), stop=False,
                                     skip_group_check=True)
                nc.tensor.matmul(gw1p[:], ones_row[:], b1row[:],
                                 start=False, stop=True, skip_group_check=True)
                gw1 = gsb.tile([GBLK, H], bf16, tag="gw1")
                nc.scalar.copy(gw1[:], gw1p[:])

                pooled = psp.tile([GBLK, H], fp32, tag="pspool")
                gtall = gtp.tile([128, NT], fp32, tag="gtall")
                ng = NT // GB

                def pool_pair(pp):
                    """eg = ech * gate ; pooled += eg^T @ x for pair pp."""
                    t0 = pp * PGB
                    n = min(PGB, NT - t0)
                    xn = xn_stream(j, t0, n, xnstr, "xnstr")
                    # one batched DVE multiply for the whole pair: the gate
                    # column broadcasts over the 32 window columns
                    eg = epool.tile([128, PGB, WG], bf16, tag="eg")
                    nc.vector.tensor_tensor(
                        eg[:, 0:n, :], ej[:, t0 : t0 + n, :],
                        gtall[:, t0 : t0 + n].unsqueeze(-1).broadcast_to(
                            [128, n, WG]),
                        op=mybir.AluOpType.mult,
                    )
                    for c in range(n):
                        t = t0 + c
                        w = t // TW
                        nc.tensor.matmul(
                            pooled[w * WG : (w + 1) * WG, :], eg[:, c, :],
                            xn[:, c, :],
                            start=(t % TW == 0), stop=(t % TW == TW - 1),
                            skip_group_check=True, tile_position=(0, w * WG),
                        )

                # gate phase: group pairs with stationary-operand reuse;
                # pooling MMs interleave LAG pairs behind so the gate tail
                # (dot -> bounce -> sigmoid) never stalls PE
                LAG = 3
                for p0 in range(0, ng, 2):
                    gis = [gi for gi in (p0, p0 + 1) if gi < ng]
                    xts_l, h1t_l = [], []
                    for gi in gis:
                        xts = xtstr.tile([128, 2, GBW], fp8, tag="xtstr")
                        nc.sync.dma_start(
                            xts[:],
                            xt8_d[j, :, :, gi * GBW : (gi + 1) * GBW],
                        )
                        xts_l.append(xts)
                        h1t_l.append(
                            psp.tile([128, 2, GBW], fp32, tag="h1t",
                                     name="h1t")
                        )
                    for h in range(2):
                        for k in range(len(gis)):
                            nc.tensor.matmul(h1t_l[k][:, h, :], w1dr[h][:],
                                             xts_l[k][:], start=True,
                                             stop=False, perf_mode=DR,
                                             skip_group_check=True)
                        for k, gi in enumerate(gis):
                            nc.tensor.matmul(
                                h1t_l[k][:, h, :],
                                gw1[:, h * 128 : (h + 1) * 128],
                                etj[:, gi * GB : (gi + 1) * GB, :],
                                start=False, stop=True,
                                skip_group_check=True,
                            )
                    for k, gi in enumerate(gis):
                        rsb = trsh.tile([128, 2, GBW], fp8, tag="trsh")
                        if k == 0:
                            # relu on Scalar for group 0 of the pair
                            nc.scalar.activation(
                                rsb[:], h1t_l[k][:],
                                mybir.ActivationFunctionType.Relu)
                        else:
                            # ... and on Vector for group 1
                            nc.vector.tensor_scalar(
                                rsb[:], h1t_l[k][:], 0.0, None,
                                op0=mybir.AluOpType.max)
                        s_ps = psp.tile([16, GB, 128], fp32, tag="psrow")
                        nc.tensor.matmul(s_ps[:], w2dr[:], rsb[:],
                                         start=True, stop=True, perf_mode=DR)
                        if k == 0:
                            g_row = growp.tile([1, 2, GB, 128], fp32,
                                               tag="grow")
                        # fused sigmoid+bias evacuation of the dots (Scalar)
                        nc.scalar.activation(
                            g_row[:, k, :, :], s_ps[0:1, :, :],
                            mybir.ActivationFunctionType.Sigmoid, bias=b2s[:])
                    # one DRAM bounce per pair (HW DMA queues) transposes
                    # both raw dot rows into columns
                    npair = len(gis)
                    gscr = gscrp.tile([npair, GB * 128], fp32, tag="gscr",
                                      name="gscr")
                    nc.gpsimd.dma_start(gscr[:], g_row[:, :npair, :, :])
                    nc.gpsimd.dma_start(
                        gtall[:, p0 * GB : (p0 + npair) * GB],
                        gscr[:].rearrange("g (c n) -> n (g c)", n=128),
                    )
                    pp = p0 // 2
                    if pp >= LAG:
                        pool_pair(pp - LAG)
                npair_tot = (ng + 1) // 2
                for pp in range(max(0, npair_tot - LAG), npair_tot):
                    pool_pair(pp)
                ps_sb = gsb.tile([GBLK, H], fp32, tag="poolsb")
                nc.scalar.copy(ps_sb[:], pooled[:])
                pf = fm_copy(ps_sb[:], gsb, "poolfm", bf16)

                # ---- GRU cell (graph-major) ----
                gf, h_old = g_fm, g_gm

                def gru_mm(psum, wi, wh, bias_row, bcol0, bn):
                    mms = []
                    if wi is not None:
                        mms += [(pf[:, ki, :], wi[ki][:, bcol0 : bcol0 + bn])
                                for ki in range(2)]
                    if wh is not None:
                        mms += [(gf[:, ki, :], wh[ki][:, bcol0 : bcol0 + bn])
                                for ki in range(2)]
                    for i, (lhsT, rhs) in enumerate(mms):
                        nc.tensor.matmul(
                            psum[:], lhsT, rhs, start=(i == 0), stop=False,
                            skip_group_check=True,
                        )
                    nc.tensor.matmul(
                        psum[:], ones_row[:], bias_row, start=False, stop=True,
                        skip_group_check=True,
                    )

                # hn first: it only needs gf (ready at ts start), so its MMs
                # can overlap the pool drain
                hnp = psp.tile([GBLK, H], fp32, tag="pspool", name="hnp")
                gru_mm(hnp, None, whh, bhn[:], 2 * H, H)
                rp = psp.tile([GBLK, H], fp32, tag="pspool", name="rp")
                gru_mm(rp, wih, whh, brz[:, 0:H], 0, H)
                r = smallsb.tile([GBLK, H], fp32, tag="gru_r")
                nc.scalar.activation(r[:], rp[:], mybir.ActivationFunctionType.Sigmoid)
                t1 = smallsb.tile([GBLK, H], fp32, tag="gru_s1")
                nc.vector.tensor_mul(t1[:], r[:], hnp[:])
                zp = psp.tile([GBLK, H], fp32, tag="pspool", name="zp")
                gru_mm(zp, wih, whh, brz[:, H : 2 * H], H, H)
                z = smallsb.tile([GBLK, H], fp32, tag="gru_z")
                nc.scalar.activation(z[:], zp[:], mybir.ActivationFunctionType.Sigmoid)
                inp_ = psp.tile([GBLK, H], fp32, tag="pspool", name="inp_")
                gru_mm(inp_, wih, None, bin_[:], 2 * H, H)
                t2 = smallsb.tile([GBLK, H], fp32, tag="gru_s2")
                nc.vector.tensor_add(t2[:], t1[:], inp_[:])
                n = smallsb.tile([GBLK, H], fp32, tag="gru_n")
                nc.scalar.activation(n[:], t2[:], mybir.ActivationFunctionType.Tanh)
                t3 = smallsb.tile([GBLK, H], fp32, tag="gru_s1")
                nc.vector.tensor_sub(t3[:], h_old[:], n[:])
                t4 = smallsb.tile([GBLK, H], fp32, tag="gru_s2")
                nc.vector.tensor_mul(t4[:], z[:], t3[:])
                t5 = smallsb.tile([GBLK, H], fp32, tag="gru_s3")
                nc.vector.tensor_add(t5[:], n[:], t4[:])
                g_gm = gsb.tile([GBLK, H], fp32, tag="gsb")
                nc.scalar.activation(g_gm[:], t5[:],
                                     mybir.ActivationFunctionType.Relu)
                if ts < NUM_TIMESTEPS - 1:
                    g_fm = fm_copy(g_gm[:], gsb, "gfm", bf16)

            nc.sync.dma_start(out_d[j * GBLK : (j + 1) * GBLK, :], g_gm[:])

    nc.compile()
    return nc


def _prep_inputs(x, batch, counts, n_cores, nblk, TW=None):
    """Host-side shard + window-pad + layout. Returns (per_core, TW)."""
    import ml_dtypes

    G = n_cores * nblk * GBLK
    NWTOT = G // WG
    batch = np.asarray(batch).astype(np.int64)
    x = np.asarray(x, dtype=np.float32)

    win_edges = np.searchsorted(batch, np.arange(0, G + 1, WG))
    win_cnt = np.diff(win_edges)
    if TW is None:
        TW = int(np.ceil(win_cnt.max() / 128))
    NT = NWIN * TW
    NTP = NT * 128
    TWP = TW * 128  # padded nodes per window

    invc_all = (1.0 / np.maximum(counts, 1.0)).astype(np.float32)

    xb = x.astype(ml_dtypes.bfloat16)
    gar = np.arange(WG, dtype=np.int64)
    per_core = []
    for k in range(n_cores):
        xk = np.zeros((nblk * NTP, H), dtype=ml_dtypes.bfloat16)
        ech = np.zeros((nblk, 128, NT, WG), dtype=ml_dtypes.bfloat16)
        etoh = np.zeros((nblk, 128, NT, 128), dtype=ml_dtypes.float8_e4m3)
        for j in range(nblk):
            bi = k * nblk + j
            lb = np.full(NTP, -1, dtype=np.int64)
            for w in range(NWIN):
                W = bi * NWIN + w
                lo, hi = win_edges[W], win_edges[W + 1]
                cnt = hi - lo
                base = j * NTP + w * TWP
                xk[base : base + cnt] = xb[lo:hi]
                lb[w * TWP : w * TWP + cnt] = batch[lo:hi] - (bi * GBLK)
            lt = lb.reshape(NT, 128)
            # window-local one-hot scaled by 1/count: ech[p, t, c]
            wof = (np.arange(NT) // TW) * WG  # window col offset per tile
            m = lt[:, :, None] == (wof[:, None, None] + gar[None, None, :])
            vals = invc_all[bi * GBLK + np.clip(lt, 0, GBLK - 1)]
            ech[j] = (m * vals[:, :, None]).transpose(1, 0, 2).astype(
                ml_dtypes.bfloat16)
            # full-block one-hot transpose for the gather matmuls
            e = (lt[:, :, None] == np.arange(GBLK)[None, None, :])
            etoh[j] = e.transpose(2, 0, 1).astype(ml_dtypes.float8_e4m3)
        # pair-interleaved feature-major fp8: [p, ki, node]
        xkT8 = np.ascontiguousarray(
            xk.reshape(nblk, NTP, 2, 128).transpose(0, 3, 2, 1)
        ).astype(ml_dtypes.float8_e4m3)
        per_core.append({"xk": xk, "xkT8": xkT8, "ech": ech, "etoh": etoh})
    return per_core, TW


def _const_inputs(gate_w1, gate_b1, gate_w2, gate_b2, gru_w_ih, gru_w_hh,
                  gru_b_ih, gru_b_hh):
    import ml_dtypes

    f = np.float32
    bf = ml_dtypes.bfloat16
    f8 = ml_dtypes.float8_e4m3
    c = {}
    w1 = np.asarray(gate_w1, f)
    c["w1t"] = np.ascontiguousarray(w1.T.reshape(2, 128, H)).astype(bf)
    # DoubleRow stationary: w1dr[h, p, i, m] = W1[h*128+m, i*128+p]
    c["w1dr"] = np.ascontiguousarray(
        w1.T.reshape(2, 128, 2, 128).transpose(2, 1, 0, 3)).astype(f8)
    c["b1row"] = np.asarray(gate_b1, f).reshape(1, H).astype(bf)
    w2p = np.zeros((128, 2, 16), f)
    w2p[:, :, 0] = np.asarray(gate_w2, f).reshape(2, 128).T
    c["w2dr"] = w2p.astype(f8)
    c["b2s"] = np.asarray(gate_b2, f).reshape(1, 1)
    c["wih_t"] = np.ascontiguousarray(
        np.asarray(gru_w_ih, f).T).reshape(2, 128, 3 * H).astype(bf)
    c["whh_t"] = np.ascontiguousarray(
        np.asarray(gru_w_hh, f).T).reshape(2, 128, 3 * H).astype(bf)
    bih = np.asarray(gru_b_ih, f)
    bhh = np.asarray(gru_b_hh, f)
    c["bsum_rz"] = (bih[: 2 * H] + bhh[: 2 * H]).reshape(1, 2 * H).astype(bf)
    c["bihn"] = bih[2 * H :].reshape(1, H).astype(bf)
    c["bhhn"] = bhh[2 * H :].reshape(1, H).astype(bf)
    c["eye128"] = np.eye(128, dtype=f)
    return c


_CACHE = {}


def run(x, gate_w1, gate_b1, gate_w2, gate_b2, gru_w_ih, gru_w_hh, gru_b_ih,
        gru_b_hh, batch, num_graphs, n_cores=8, nblk=NBLK, trace=False,
        use_sim=False):
    from concourse.bass_utils import run_bass_kernel_spmd

    batch = np.asarray(batch).astype(np.int64)
    G = n_cores * nblk * GBLK
    counts = np.bincount(batch, minlength=G).astype(np.float32)
    per_core, TW = _prep_inputs(x, batch, counts, n_cores, nblk)
    consts = _const_inputs(gate_w1, gate_b1, gate_w2, gate_b2, gru_w_ih,
                           gru_w_hh, gru_b_ih, gru_b_hh)
    in_maps = [{**consts, **pc} for pc in per_core]

    key = (TW, nblk, n_cores)
    if key not in _CACHE:
        _CACHE[key] = _build_program(TW, nblk=nblk)
    nc = _CACHE[key]

    if use_sim:
        from concourse.bass_interp import CoreSim

        outs = []
        for k in range(n_cores):
            sim = CoreSim(nc)
            for name, arr in in_maps[k].items():
                sim.tensor(name)[:] = arr
            sim.simulate()
            outs.append(np.array(sim.tensor("out")))
        return np.concatenate(outs, axis=0), None

    res = run_bass_kernel_spmd(nc, in_maps, core_ids=list(range(n_cores)),
                               trace=trace)
    out = np.concatenate([res.results[k]["out"] for k in range(n_cores)], axis=0)
    return out, res


def kernel(**inputs):
    out, _ = run(**inputs)
    return out


# revision 33
# speedup vs baseline: 1.3004x; 1.0346x over previous
"""Trainium2 Bass kernel for AttentiveGraphPooling (gnn_message_passing).

Strategy: shard the 4096 graphs across 8 cores (512 graphs each). batch is
sorted, so each core owns a contiguous node range covering whole graphs ->
pooling / gather / GRU are all core-local, no collectives needed.

Per core, graphs go in 4 blocks of 128, each block in 4 windows of 32
graphs. The host pads each window's nodes to a fixed TW node-tiles so the
program is uniform across cores. Pooling uses a [128, 32] gate-weighted
one-hot stationary per node tile (host-prescaled by 1/count so the PSUM
accumulation directly yields the mean); each window accumulates into its
own 32-partition strip of the [128, 256] pooled PSUM tile.

The gate MLP runs feature-major so every matmul has a *block-constant*
stationary operand:
  h1T[f, n] = W1 @ (x + g_b)^T = (fp8 DoubleRow: W1-pairs stationary, x^T
  pair-interleaved moving, N=512 per 4-tile group) + (gather: gw1
  graph-major half-columns stationary, E^T fp8 moving).
relu(h1T) -> SBUF fp8 (split Scalar/Vector by group parity), then the
gate dot w2 . relu(h1T) runs on the PE (w2 half-columns stationary,
N=512) into a [1, 512] PSUM row; a fused sigmoid+bias activation
(Scalar) evacuates it to SBUF; a DRAM bounce (gpsimd-executed DMA)
transposes the gate rows into per-tile columns. Weighted pooling is
eg^T @ x on the PE with the 32-wide stationary (eg batched per 8-tile
chunk on DVE via a broadcast multiply). The GRU runs per graph-block
(hn-first so its matmuls overlap the pool drain) with bf16 K=1 bias
matmuls; block j+1's phase A is issued at block j's ts0 start so its
DMA-paced pool matmuls fill PE drain gaps.
"""

import os
import sys

import numpy as np

sys.path.insert(0, "/opt/trn_rl_repo")

H = 256
NBLK = 4  # graph blocks per core
GBLK = 128  # graphs per block
NWIN = 4  # windows per block (32 graphs each)
WG = 32  # graphs per window
NUM_TIMESTEPS = 2
GB = 4  # node tiles per gate-pipeline group
PGB = 8  # node tiles per streamed node-major x group


def _build_program(TW, nblk=NBLK):
    """Build the single-core SPMD Bass program. TW = node tiles per window."""
    from contextlib import ExitStack

    import concourse.bass as bass
    import concourse.tile as tile
    from concourse import bacc, mybir

    fp32 = mybir.dt.float32
    bf16 = mybir.dt.bfloat16
    fp8 = mybir.dt.float8e4
    DR = mybir.MatmulPerfMode.DoubleRow

    NT = NWIN * TW  # node tiles per block
    NTP = NT * 128  # padded nodes per block
    GBW = GB * 128

    nc = bacc.Bacc("TRN2", target_bir_lowering=False, debug=False)

    # ---- DRAM parameters (per-core inputs) ----
    x_d = nc.dram_tensor("xk", [nblk * NTP, H], bf16, kind="ExternalInput")
    xt8_d = nc.dram_tensor("xkT8", [nblk, 128, 2, NTP], fp8, kind="ExternalInput")
    e_d = nc.dram_tensor("ech", [nblk, 128, NT, WG], bf16, kind="ExternalInput")
    etj_d = nc.dram_tensor("etoh", [nblk, 128, NT, 128], fp8, kind="ExternalInput")
    w1t_d = nc.dram_tensor("w1t", [2, 128, H], bf16, kind="ExternalInput")
    w1dr_d = nc.dram_tensor("w1dr", [2, 128, 2, 128], fp8, kind="ExternalInput")
    b1r_d = nc.dram_tensor("b1row", [1, H], bf16, kind="ExternalInput")
    w2c_d = nc.dram_tensor("w2dr", [128, 2, 16], fp8, kind="ExternalInput")
    b2c_d = nc.dram_tensor("b2s", [1, 1], fp32, kind="ExternalInput")
    wih_d = nc.dram_tensor("wih_t", [2, 128, 3 * H], bf16, kind="ExternalInput")
    whh_d = nc.dram_tensor("whh_t", [2, 128, 3 * H], bf16, kind="ExternalInput")
    brz_d = nc.dram_tensor("bsum_rz", [1, 2 * H], bf16, kind="ExternalInput")
    bin_d = nc.dram_tensor("bihn", [1, H], bf16, kind="ExternalInput")
    bhn_d = nc.dram_tensor("bhhn", [1, H], bf16, kind="ExternalInput")
    eye_d = nc.dram_tensor("eye128", [128, 128], fp32, kind="ExternalInput")
    out_d = nc.dram_tensor("out", [nblk * GBLK, H], fp32, kind="ExternalOutput")

    with tile.TileContext(nc) as tc, ExitStack() as ctx:
        ep = ctx.enter_context  # shorthand

        const = ep(tc.tile_pool(name="const", bufs=1))
        eres = ep(tc.tile_pool(name="eres", bufs=2))
        etres = ep(tc.tile_pool(name="etres", bufs=2))
        xtstr = ep(tc.tile_pool(name="xtstr", bufs=6))
        xnstr = ep(tc.tile_pool(name="xnstr", bufs=10))
        xastr = ep(tc.tile_pool(name="xastr", bufs=8))
        epool = ep(tc.tile_pool(name="egpool", bufs=10))
        trsh = ep(tc.tile_pool(name="trsh", bufs=5))
        growp = ep(tc.tile_pool(name="grow", bufs=3))
        gtp = ep(tc.tile_pool(name="gtp", bufs=2))
        gsb = ep(tc.tile_pool(name="gsb", bufs=3))
        smallsb = ep(tc.tile_pool(name="smallsb", bufs=2))

        gscrp = ep(tc.tile_pool(name="gscr", bufs=4, space="DRAM"))

        psp = ep(tc.tile_pool(name="psp", bufs=2, space="PSUM"))

        # ---- load constants ----
        def cload(shape, src, tag, dt=fp32):
            t = const.tile(shape, dt, tag=tag)
            nc.sync.dma_start(t[:], src)
            return t

        eye = cload([128, 128], eye_d[:], "c_eye")
        w1t = [cload([128, H], w1t_d[k], f"c_w1t{k}", bf16) for k in range(2)]
        w1dr = [cload([128, 2, 128], w1dr_d[h], f"c_w1dr{h}", fp8)
                for h in range(2)]
        b1row = cload([1, H], b1r_d[:], "c_b1r", bf16)
        w2dr = cload([128, 2, 16], w2c_d[:], "c_w2dr", fp8)
        b2s = cload([1, 1], b2c_d[:], "c_b2s")
        wih = [cload([128, 3 * H], wih_d[k], f"c_wih{k}", bf16) for k in range(2)]
        whh = [cload([128, 3 * H], whh_d[k], f"c_whh{k}", bf16) for k in range(2)]
        brz = cload([1, 2 * H], brz_d[:], "c_brz", bf16)
        bin_ = cload([1, H], bin_d[:], "c_bin", bf16)
        bhn = cload([1, H], bhn_d[:], "c_bhn", bf16)
        ones_row = const.tile([1, 128], bf16)
        nc.vector.memset(ones_row[:], 1.0)

        def fm_copy(g_ap, pool, tag, dt):
            """(128,256) graph-major -> feature-major (128,2,128) via PE."""
            gf = pool.tile([128, 2, GBLK], dt, tag=tag)
            for ki in range(2):
                tp = psp.tile([128, 128], fp32, tag="psrow", name="tp")
                nc.tensor.matmul(tp[:], g_ap[:, ki * 128 : (ki + 1) * 128], eye[:],
                                 is_transpose=True, start=True, stop=True)
                nc.scalar.copy(gf[:, ki, :], tp[:])
            return gf

        blocks = []

        def load_block(j):
            """DMA one-hot chunks for block j."""
            ej = eres.tile([128, NT, WG], bf16, tag="eres", name="ej")
            nc.sync.dma_start(ej[:], e_d[j])
            etj = etres.tile([128, NT, 128], fp8, tag="etres", name="etj")
            nc.sync.dma_start(etj[:], etj_d[j])
            return ej, etj

        def xn_stream(j, t0, n, pool, tag):
            """Stream node-major x tiles [t0, t0+n) of block j."""
            xn = pool.tile([128, PGB, H], bf16, tag=tag, name="xn")
            base = j * NTP + t0 * 128
            nc.sync.dma_start(
                xn[:, 0:n, :], x_d[base : base + n * 128, :].rearrange(
                    "(c p) h -> p c h", p=128
                )
            )
            return xn

        blocks.append(load_block(0))
        for j in range(nblk):
            ej, etj = blocks[j]

            def phase_a(j, ej):
                """Initial mean pool for block j -> (g_gm, g_fm)."""
                pooled = psp.tile([GBLK, H], fp32, tag="pspool")
                for t0 in range(0, NT, PGB):
                    na = min(PGB, NT - t0)
                    xn = xn_stream(j, t0, na, xastr, "xastr")
                    for c in range(na):
                        t = t0 + c
                        w = t // TW
                        nc.tensor.matmul(
                            pooled[w * WG : (w + 1) * WG, :],
                            ej[:, t, :], xn[:, c, :],
                            start=(t % TW == 0), stop=(t % TW == TW - 1),
                            skip_group_check=True, tile_position=(0, w * WG),
                        )
                g_gm = gsb.tile([GBLK, H], fp32, tag="gsb")
                nc.scalar.copy(g_gm[:], pooled[:])
                return g_gm, fm_copy(g_gm[:], gsb, "gfm", bf16)

            if j == 0:
                g_gm, g_fm = phase_a(0, ej)
            else:
                g_gm, g_fm = ga_next

            # ---- timesteps ----
            for ts in range(NUM_TIMESTEPS):
                if ts == 0 and j + 1 < nblk:
                    # prefetch next block's one-hots and issue its phase A
                    # up front: its pool matmuls are DMA-paced, so they
                    # trickle into this block's PE drain gaps, and the
                    # pspool slot rotation leaves hnp an early slot
                    blocks.append(load_block(j + 1))
                    ga_next = phase_a(j + 1, blocks[j + 1][0])
                # GW1 = G @ W1^T + b1  (graph-major, bf16) — also the
                # stationary operand of the gather matmuls
                gw1p = psp.tile([GBLK, H], fp32, tag="pspool", name="gw1p")
                for ki in range(2):
                    nc.tensor.matmul(gw1p[:], g_fm[:, ki, :], w1t[ki][:],
                                     start=(ki == 0), stop=False,
                                     skip_group_check=True)
                nc.tensor.matmul(gw1p[:], ones_row[:], b1row[:],
                                 start=False, stop=True, skip_group_check=True)
                gw1 = gsb.tile([GBLK, H], bf16, tag="gw1")
                nc.scalar.copy(gw1[:], gw1p[:])

                pooled = psp.tile([GBLK, H], fp32, tag="pspool")
                gtall = gtp.tile([128, NT], fp32, tag="gtall")
                ng = NT // GB

                def pool_pair(pp):
                    """eg = ech * gate ; pooled += eg^T @ x for pair pp."""
                    t0 = pp * PGB
                    n = min(PGB, NT - t0)
                    xn = xn_stream(j, t0, n, xnstr, "xnstr")
                    # one batched DVE multiply for the whole pair: the gate
                    # column broadcasts over the 32 window columns
                    eg = epool.tile([128, PGB, WG], bf16, tag="eg")
                    nc.vector.tensor_tensor(
                        eg[:, 0:n, :], ej[:, t0 : t0 + n, :],
                        gtall[:, t0 : t0 + n].unsqueeze(-1).broadcast_to(
                            [128, n, WG]),
                        op=mybir.AluOpType.mult,
                    )
                    for c in range(n):
                        t = t0 + c
                        w = t // TW
                        nc.tensor.matmul(
                            pooled[w * WG : (w + 1) * WG, :], eg[:, c, :],
                            xn[:, c, :],
                            start=(t % TW == 0), stop=(t % TW == TW - 1),
                            skip_group_check=True, tile_position=(0, w * WG),
                        )

                # gate phase: group pairs with stationary-operand reuse;
                # pooling MMs interleave LAG pairs behind so the gate tail
                # (dot -> bounce -> sigmoid) never stalls PE
                LAG = 3
                for p0 in range(0, ng, 2):
                    gis = [gi for gi in (p0, p0 + 1) if gi < ng]
                    xts_l, h1t_l = [], []
                    for gi in gis:
                        xts = xtstr.tile([128, 2, GBW], fp8, tag="xtstr")
                        nc.sync.dma_start(
                            xts[:],
                            xt8_d[j, :, :, gi * GBW : (gi + 1) * GBW],
                        )
                        xts_l.append(xts)
                        h1t_l.append(
                            [psp.tile([128, GBW], fp32, tag="h1t",
                                      name="h1t", bufs=4) for _ in range(2)]
                        )
                    for h in range(2):
                        for k in range(len(gis)):
                            nc.tensor.matmul(h1t_l[k][h][:], w1dr[h][:],
                                             xts_l[k][:], start=True,
                                             stop=False, perf_mode=DR,
                                             skip_group_check=True)
                        for k, gi in enumerate(gis):
                            nc.tensor.matmul(
                                h1t_l[k][h][:],
                                gw1[:, h * 128 : (h + 1) * 128],
                                etj[:, gi * GB : (gi + 1) * GB, :],
                                start=False, stop=True,
                                skip_group_check=True,
                            )
                    for k, gi in enumerate(gis):
                        rsb = trsh.tile([128, 2, GBW], fp8, tag="trsh")
                        # per-half relu on different engines: the halves
                        # finish in parallel, halving the w2 wait, and the
                        # 1-bank h1t tiles rotate 4-deep
                        nc.scalar.activation(
                            rsb[:, 0, :], h1t_l[k][0][:],
                            mybir.ActivationFunctionType.Relu)
                        nc.vector.tensor_scalar(
                            rsb[:, 1, :], h1t_l[k][1][:], 0.0, None,
                            op0=mybir.AluOpType.max)
                        s_ps = psp.tile([16, GB, 128], fp32, tag="psrow")
                        nc.tensor.matmul(s_ps[:], w2dr[:], rsb[:],
                                         start=True, stop=True, perf_mode=DR)
                        if k == 0:
                            g_row = growp.tile([1, 2, GB, 128], fp32,
                                               tag="grow")
                        # fused sigmoid+bias evacuation of the dots (Scalar)
                        nc.scalar.activation(
                            g_row[:, k, :, :], s_ps[0:1, :, :],
                            mybir.ActivationFunctionType.Sigmoid, bias=b2s[:])
                    # one DRAM bounce per pair (HW DMA queues) transposes
                    # both raw dot rows into columns
                    npair = len(gis)
                    gscr = gscrp.tile([npair, GB * 128], fp32, tag="gscr",
                                      name="gscr")
                    nc.gpsimd.dma_start(gscr[:], g_row[:, :npair, :, :])
                    nc.gpsimd.dma_start(
                        gtall[:, p0 * GB : (p0 + npair) * GB],
                        gscr[:].rearrange("g (c n) -> n (g c)", n=128),
                    )
                    pp = p0 // 2
                    if pp >= LAG:
                        pool_pair(pp - LAG)
                npair_tot = (ng + 1) // 2
                for pp in range(max(0, npair_tot - LAG), npair_tot):
                    pool_pair(pp)
                ps_sb = gsb.tile([GBLK, H], fp32, tag="poolsb")
                nc.scalar.copy(ps_sb[:], pooled[:])
                pf = fm_copy(ps_sb[:], gsb, "poolfm", bf16)

                # ---- GRU cell (graph-major) ----
                gf, h_old = g_fm, g_gm

                def gru_mm(psum, wi, wh, bias_row, bcol0, bn):
                    mms = []
                    if wi is not None:
                        mms += [(pf[:, ki, :], wi[ki][:, bcol0 : bcol0 + bn])
                                for ki in range(2)]
                    if wh is not None:
                        mms += [(gf[:, ki, :], wh[ki][:, bcol0 : bcol0 + bn])
                                for ki in range(2)]
                    for i, (lhsT, rhs) in enumerate(mms):
                        nc.tensor.matmul(
                            psum[:], lhsT, rhs, start=(i == 0), stop=False,
                            skip_group_check=True,
                        )
                    nc.tensor.matmul(
                        psum[:], ones_row[:], bias_row, start=False, stop=True,
                        skip_group_check=True,
                    )

                # hn first: it only needs gf (ready at ts start), so its MMs
                # can overlap the pool drain
                hnp = psp.tile([GBLK, H], fp32, tag="pspool", name="hnp")
                gru_mm(hnp, None, whh, bhn[:], 2 * H, H)
                rp = psp.tile([GBLK, H], fp32, tag="pspool", name="rp")
                gru_mm(rp, wih, whh, brz[:, 0:H], 0, H)
                r = smallsb.tile([GBLK, H], fp32, tag="gru_r")
                nc.scalar.activation(r[:], rp[:], mybir.ActivationFunctionType.Sigmoid)
                t1 = smallsb.tile([GBLK, H], fp32, tag="gru_s1")
                nc.vector.tensor_mul(t1[:], r[:], hnp[:])
                zp = psp.tile([GBLK, H], fp32, tag="pspool", name="zp")
                gru_mm(zp, wih, whh, brz[:, H : 2 * H], H, H)
                z = smallsb.tile([GBLK, H], fp32, tag="gru_z")
                nc.scalar.activation(z[:], zp[:], mybir.ActivationFunctionType.Sigmoid)
                inp_ = psp.tile([GBLK, H], fp32, tag="pspool", name="inp_")
                gru_mm(inp_, wih, None, bin_[:], 2 * H, H)
                t2 = smallsb.tile([GBLK, H], fp32, tag="gru_s2")
                nc.vector.tensor_add(t2[:], t1[:], inp_[:])
                n = smallsb.tile([GBLK, H], fp32, tag="gru_n")
                nc.scalar.activation(n[:], t2[:], mybir.ActivationFunctionType.Tanh)
                t3 = smallsb.tile([GBLK, H], fp32, tag="gru_s1")
                nc.vector.tensor_sub(t3[:], h_old[:], n[:])
                t4 = smallsb.tile([GBLK, H], fp32, tag="gru_s2")
                nc.vector.tensor_mul(t4[:], z[:], t3[:])
                t5 = smallsb.tile([GBLK, H], fp32, tag="gru_s3")
                nc.vector.tensor_add(t5[:], n[:], t4[:])
                g_gm = gsb.tile([GBLK, H], fp32, tag="gsb")
                nc.scalar.activation(g_gm[:], t5[:],
                                     mybir.ActivationFunctionType.Relu)
                if ts < NUM_TIMESTEPS - 1:
                    g_fm = fm_copy(g_gm[:], gsb, "gfm", bf16)

            nc.sync.dma_start(out_d[j * GBLK : (j + 1) * GBLK, :], g_gm[:])

    nc.compile()
    return nc


def _prep_inputs(x, batch, counts, n_cores, nblk, TW=None):
    """Host-side shard + window-pad + layout. Returns (per_core, TW)."""
    import ml_dtypes

    G = n_cores * nblk * GBLK
    NWTOT = G // WG
    batch = np.asarray(batch).astype(np.int64)
    x = np.asarray(x, dtype=np.float32)

    win_edges = np.searchsorted(batch, np.arange(0, G + 1, WG))
    win_cnt = np.diff(win_edges)
    if TW is None:
        TW = int(np.ceil(win_cnt.max() / 128))
    NT = NWIN * TW
    NTP = NT * 128
    TWP = TW * 128  # padded nodes per window

    invc_all = (1.0 / np.maximum(counts, 1.0)).astype(np.float32)

    xb = x.astype(ml_dtypes.bfloat16)
    gar = np.arange(WG, dtype=np.int64)
    per_core = []
    for k in range(n_cores):
        xk = np.zeros((nblk * NTP, H), dtype=ml_dtypes.bfloat16)
        ech = np.zeros((nblk, 128, NT, WG), dtype=ml_dtypes.bfloat16)
        etoh = np.zeros((nblk, 128, NT, 128), dtype=ml_dtypes.float8_e4m3)
        for j in range(nblk):
            bi = k * nblk + j
            lb = np.full(NTP, -1, dtype=np.int64)
            for w in range(NWIN):
                W = bi * NWIN + w
                lo, hi = win_edges[W], win_edges[W + 1]
                cnt = hi - lo
                base = j * NTP + w * TWP
                xk[base : base + cnt] = xb[lo:hi]
                lb[w * TWP : w * TWP + cnt] = batch[lo:hi] - (bi * GBLK)
            lt = lb.reshape(NT, 128)
            # window-local one-hot scaled by 1/count: ech[p, t, c]
            wof = (np.arange(NT) // TW) * WG  # window col offset per tile
            m = lt[:, :, None] == (wof[:, None, None] + gar[None, None, :])
            vals = invc_all[bi * GBLK + np.clip(lt, 0, GBLK - 1)]
            ech[j] = (m * vals[:, :, None]).transpose(1, 0, 2).astype(
                ml_dtypes.bfloat16)
            # full-block one-hot transpose for the gather matmuls
            e = (lt[:, :, None] == np.arange(GBLK)[None, None, :])
            etoh[j] = e.transpose(2, 0, 1).astype(ml_dtypes.float8_e4m3)
        # pair-interleaved feature-major fp8: [p, ki, node]
        xkT8 = np.ascontiguousarray(
            xk.reshape(nblk, NTP, 2, 128).transpose(0, 3, 2, 1)
        ).astype(ml_dtypes.float8_e4m3)
        per_core.append({"xk": xk, "xkT8": xkT8, "ech": ech, "etoh": etoh})
    return per_core, TW


def _const_inputs(gate_w1, gate_b1, gate_w2, gate_b2, gru_w_ih, gru_w_hh,
                  gru_b_ih, gru_b_hh):
    import ml_dtypes

    f = np.float32
    bf = ml_dtypes.bfloat16
    f8 = ml_dtypes.float8_e4m3
    c = {}
    w1 = np.asarray(gate_w1, f)
    c["w1t"] = np.ascontiguousarray(w1.T.reshape(2, 128, H)).astype(bf)
    # DoubleRow stationary: w1dr[h, p, i, m] = W1[h*128+m, i*128+p]
    c["w1dr"] = np.ascontiguousarray(
        w1.T.reshape(2, 128, 2, 128).transpose(2, 1, 0, 3)).astype(f8)
    c["b1row"] = np.asarray(gate_b1, f).reshape(1, H).astype(bf)
    w2p = np.zeros((128, 2, 16), f)
    w2p[:, :, 0] = np.asarray(gate_w2, f).reshape(2, 128).T
    c["w2dr"] = w2p.astype(f8)
    c["b2s"] = np.asarray(gate_b2, f).reshape(1, 1)
    c["wih_t"] = np.ascontiguousarray(
        np.asarray(gru_w_ih, f).T).reshape(2, 128, 3 * H).astype(bf)
    c["whh_t"] = np.ascontiguousarray(
        np.asarray(gru_w_hh, f).T).reshape(2, 128, 3 * H).astype(bf)
    bih = np.asarray(gru_b_ih, f)
    bhh = np.asarray(gru_b_hh, f)
    c["bsum_rz"] = (bih[: 2 * H] + bhh[: 2 * H]).reshape(1, 2 * H).astype(bf)
    c["bihn"] = bih[2 * H :].reshape(1, H).astype(bf)
    c["bhhn"] = bhh[2 * H :].reshape(1, H).astype(bf)
    c["eye128"] = np.eye(128, dtype=f)
    return c


_CACHE = {}


def run(x, gate_w1, gate_b1, gate_w2, gate_b2, gru_w_ih, gru_w_hh, gru_b_ih,
        gru_b_hh, batch, num_graphs, n_cores=8, nblk=NBLK, trace=False,
        use_sim=False):
    from concourse.bass_utils import run_bass_kernel_spmd

    batch = np.asarray(batch).astype(np.int64)
    G = n_cores * nblk * GBLK
    counts = np.bincount(batch, minlength=G).astype(np.float32)
    per_core, TW = _prep_inputs(x, batch, counts, n_cores, nblk)
    consts = _const_inputs(gate_w1, gate_b1, gate_w2, gate_b2, gru_w_ih,
                           gru_w_hh, gru_b_ih, gru_b_hh)
    in_maps = [{**consts, **pc} for pc in per_core]

    key = (TW, nblk, n_cores)
    if key not in _CACHE:
        _CACHE[key] = _build_program(TW, nblk=nblk)
    nc = _CACHE[key]

    if use_sim:
        from concourse.bass_interp import CoreSim

        outs = []
        for k in range(n_cores):
            sim = CoreSim(nc)
            for name, arr in in_maps[k].items():
                sim.tensor(name)[:] = arr
            sim.simulate()
            outs.append(np.array(sim.tensor("out")))
        return np.concatenate(outs, axis=0), None

    res = run_bass_kernel_spmd(nc, in_maps, core_ids=list(range(n_cores)),
                               trace=trace)
    out = np.concatenate([res.results[k]["out"] for k in range(n_cores)], axis=0)
    return out, res


def kernel(**inputs):
    out, _ = run(**inputs)
    return out
